# revision 31
# baseline (speedup 1.0000x reference)
"""Trainium2 Bass kernel for nn_LinearMultiheadAttention (linear attention with
polynomial feature map phi(x) = [1, x, 0.5 x^2]), sharded over 8 NeuronCores.

Sharding: core c -> batch b = c//2, heads h0 = (c%2)*8 .. h0+8.
Each core computes a partial output (its 8 heads' contribution through Wo);
the host sums the two partials per batch.

Precision: the z = qsum*ksum normalizer is catastrophically ill-conditioned
(min |qsum| ~3e-4 while outputs reach 6e5), so q/k projections are computed
to full fp32 accuracy via an exact 3-term fp32r split
(hs = hi + lo, W = Whi + Wlo, 12+12 mantissa bits, products exact in the
fp32 PSUM accumulate; only the lo*lo term ~2^-24 is dropped). qsum/ksum are
accumulated in exact fp32. The v / kv / qkv / Wo path is bf16.

v2 scheduling: kv matmuls deferred one tile (no PE head-of-line block on the
phi_k build), no gpsimd anywhere (measured ~20x below spec), matmul-based
ksum assembly in mid (no SBUF-SBUF DMAs / transposes), hs prefetched before
the big weight DMAs, pass B software-pipelined 3 deep with single-bank bf16
transpose targets.
"""
import numpy as np
import ml_dtypes

import concourse.bass as bass
import concourse.tile as tile
from concourse import bacc, mybir
from concourse.bass_utils import run_bass_kernel_spmd

F32 = mybir.dt.float32
F32R = mybir.dt.float32r
BF16 = mybir.dt.bfloat16

B, S, D = 4, 4096, 1040
H, F, E = 16, 32, 65          # heads, feature_dim, head_dim (= 2F+1)
HPC = 8                        # heads per core
P = 128
NT = S // P                    # 32 token tiles per core
NCH = 9                        # ceil(D/128); last chunk K=16
KLAST = D - 8 * P              # 16
QW = HPC * F                   # 256 q (or k) cols per core
VW = HPC * E                   # 520 v cols per core
VH = 4 * E                     # 260
OCH = 5                        # ceil(VW/128); last chunk K=8
OLAST = VW - 4 * P             # 8

_CACHED = {}


def _chunk_k(c):
    return KLAST if c == NCH - 1 else P


def build_bass():
    nc = bacc.Bacc("TRN2", target_bir_lowering=False, debug=False, num_devices=8)
    hs = nc.dram_tensor("hs", [S, D], F32, kind="ExternalInput").ap()
    maskf = nc.dram_tensor("maskf", [P, NT], F32, kind="ExternalInput").ap()
    wqk = nc.dram_tensor("wqk", [NCH, P, 2 * QW], F32, kind="ExternalInput").ap()
    wv = nc.dram_tensor("wv", [NCH, P, VW], BF16, kind="ExternalInput").ap()
    wo = nc.dram_tensor("wo", [OCH, P, D], BF16, kind="ExternalInput").ap()
    id32 = nc.dram_tensor("id32", [P, P], F32, kind="ExternalInput").ap()
    id16 = nc.dram_tensor("id16", [P, P], BF16, kind="ExternalInput").ap()
    out = nc.dram_tensor("out", [S, D], F32, kind="ExternalOutput").ap()

    ACT = mybir.ActivationFunctionType.Copy

    with tile.TileContext(nc) as tc:
        with (
            tc.tile_pool(name="consts", bufs=1) as consts,
            tc.tile_pool(name="state", bufs=1) as state,
            tc.tile_pool(name="rot", bufs=2) as rot,
            tc.tile_pool(name="ps", bufs=1, space="PSUM") as ps,
            tc.tile_pool(name="ps2", bufs=2, space="PSUM") as ps2,
        ):
            # ---- small consts + hs prefetch BEFORE the big weight DMAs ----
            id32_sb = consts.tile([P, P], F32)
            nc.sync.dma_start(out=id32_sb, in_=id32)
            hs_tiles = [consts.tile([P, D], F32, name=f"hst{i}")
                        for i in range(3)]
            nc.sync.dma_start(out=hs_tiles[0], in_=hs[0:P, :])
            id16_sb = consts.tile([P, P], BF16)
            nc.sync.dma_start(out=id16_sb, in_=id16)
            mask_sb = consts.tile([P, NT], F32)
            nc.sync.dma_start(out=mask_sb, in_=maskf)

            # weights as per-chunk tiles: independent deps, so tile 0's
            # matmuls start as soon as chunk 0's DMA + split land
            wqk_sb = [consts.tile([P, 2 * QW], F32, name=f"wqk{c}")
                      for c in range(NCH)]
            wqkr = [consts.tile([P, 2 * QW], F32R, name=f"wqkr{c}")
                    for c in range(NCH)]
            wqkl = [consts.tile([P, 2 * QW], F32R, name=f"wqkl{c}")
                    for c in range(NCH)]
            wv_sb = [consts.tile([P, VW], BF16, name=f"wv{c}")
                     for c in range(NCH)]
            for c in range(NCH):
                nc.sync.dma_start(out=wqk_sb[c], in_=wqk[c])
                nc.sync.dma_start(out=wv_sb[c], in_=wv[c])
                if c in (1, 2):
                    nc.sync.dma_start(out=hs_tiles[c],
                                      in_=hs[c * P:(c + 1) * P, :])
                nc.vector.tensor_copy(wqkr[c][:], wqk_sb[c][:])
                nc.vector.tensor_tensor(wqkl[c][:], wqk_sb[c][:],
                                        wqkr[c][:].bitcast(F32),
                                        mybir.AluOpType.subtract)
            wo_sb = consts.tile([P, OCH, D], BF16)
            nc.sync.dma_start(out=wo_sb, in_=wo.rearrange("c p j -> p c j"))

            half_col = consts.tile([P, 1], F32)
            nc.vector.memset(half_col[:], 0.5)
            ones_row = consts.tile([1, P], F32)
            nc.vector.memset(ones_row[:], 1.0)

            # ---- persistent state ----
            phiq = state.tile([P, NT, HPC, E], BF16)   # rq-folded phi(q) stash
            kvs_sb = state.tile([E, HPC * E], BF16)    # rksum-scaled kv
            rk_row = state.tile([1, HPC * E], F32)
            # exact per-chunk row-sums of hs^T (for the exact ksum-linear path)
            hsTsum = state.tile([P, NCH], F32)
            ksq_acc = state.tile([P, QW], F32)
            kv_ps = [ps.tile([E, VH], F32, tag=f"kv{i}", name=f"kv{i}")
                     for i in range(2)]
            # phik/v16 live one extra tile (kv deferral) -> explicit 2 bufs
            phik_t = [state.tile([P, HPC, E], BF16, name=f"phik{i}")
                      for i in range(2)]
            v16_t = [state.tile([P, VW], BF16, name=f"v16{i}")
                     for i in range(2)]
            # ones column of phi_k never changes: set once per buffer
            for i in range(2):
                nc.vector.memset(phik_t[i][:, :, 0:1], 1.0)

            def kv_mm(t):
                pk, vv = phik_t[t % 2], v16_t[t % 2]
                for h in range(HPC):
                    nc.tensor.matmul(
                        kv_ps[h // 4][:, (h % 4) * E:(h % 4) * E + E],
                        pk[:, h, :], vv[:, h * E:h * E + E],
                        start=(t == 0 and h % 4 == 0), stop=(t == NT - 1),
                        skip_group_check=True)

            # =============== PASS A ===============
            for t in range(NT):
                with nc.named_scope(f"A{t}"):
                    hs_t = hs_tiles[t % 3]

                    GW = (4, 4, 1)
                    hsr = [rot.tile([P, GW[g], P], F32R, tag=f"hsr{g}",
                                    name=f"hsr{g}_{t}") for g in range(3)]
                    hlo = [rot.tile([P, GW[g], P], F32R, tag=f"hlo{g}",
                                    name=f"hlo{g}_{t}") for g in range(3)]
                    hsT16 = [rot.tile([P, GW[g], P], BF16, tag=f"hsT16{g}",
                                      name=f"hsT16{g}_{t}") for g in range(3)]
                    for g, cs in enumerate([range(0, 4), range(4, 8), range(8, 9)]):
                        tp = ps2.tile([P, 512], F32, tag="tps",
                                      name=f"tp_{t}_{g}")
                        for c in cs:
                            kk = _chunk_k(c)
                            nc.tensor.transpose(
                                tp[0:kk, (c % 4) * P:(c % 4) * P + P],
                                hs_t[:, c * P:c * P + kk],
                                id32_sb[:])
                        lo, hi = cs[0], cs[-1] + 1
                        kk = _chunk_k(hi - 1)
                        w = (hi - 1 - lo) * P + P
                        src = tp[0:kk, 0:w]
                        hr = hsr[g][0:kk].rearrange("p c n -> p (c n)")
                        nc.scalar.activation(hr, src, ACT)
                        nc.vector.tensor_tensor(
                            hlo[g][0:kk].rearrange("p c n -> p (c n)"),
                            src, hr.bitcast(F32), mybir.AluOpType.subtract)
                        nc.scalar.activation(
                            hsT16[g][0:kk].rearrange("p c n -> p (c n)"),
                            src, ACT)
                        # exact hs^T row-sums (fp32) for the ksum-linear path
                        nch = hi - lo
                        red = rot.tile([P, 4], F32, tag="hred", bufs=3,
                                       name=f"red_{t}_{g}")
                        nc.vector.tensor_reduce(
                            red[0:kk, 0:nch],
                            tp[0:kk, 0:w].rearrange("p (c n) -> p c n", n=P),
                            mybir.AxisListType.X, mybir.AluOpType.add)
                        if t == 0:
                            nc.vector.tensor_copy(hsTsum[0:kk, lo:hi],
                                                  red[0:kk, 0:nch])
                        else:
                            nc.vector.tensor_add(hsTsum[0:kk, lo:hi],
                                                 hsTsum[0:kk, lo:hi],
                                                 red[0:kk, 0:nch])

                    # refill this hs slot for tile t+3 (readers above queued)
                    if t + 3 < NT:
                        nc.sync.dma_start(
                            out=hs_t, in_=hs[(t + 3) * P:(t + 4) * P, :])

                    # projections: q|k via exact 3-term fp32r, v via bf16
                    qk_ps = ps2.tile([P, 2 * QW], F32, tag="qk", name=f"qk_{t}")
                    v1_ps = ps.tile([P, VH], F32, tag="v1", name=f"v1_{t}")
                    v2_ps = ps.tile([P, VH], F32, tag="v2", name=f"v2_{t}")
                    # r-pass covers q|k (512); l/lo correction passes cover
                    # only the q half (256): k's error propagates relatively
                    # through 1/ksum (no catastrophic cancellation there).
                    for c in range(NCH):
                        kk = _chunk_k(c)
                        if c < NCH - 1:
                            nc.tensor.matmul(
                                qk_ps[:], hsr[c // 4][0:kk, c % 4, :], wqkr[c][0:kk, :],
                                start=(c == 0), stop=False,
                                skip_group_check=True)
                        else:
                            # split last chunk so the k region gets its stop
                            nc.tensor.matmul(
                                qk_ps[:, 0:QW], hsr[c // 4][0:kk, c % 4, :],
                                wqkr[c][0:kk, 0:QW],
                                start=False, stop=False,
                                skip_group_check=True)
                            nc.tensor.matmul(
                                qk_ps[:, QW:2 * QW], hsr[c // 4][0:kk, c % 4, :],
                                wqkr[c][0:kk, QW:2 * QW],
                                start=False, stop=True,
                                skip_group_check=True)
                        nc.tensor.matmul(
                            v1_ps[:], hsT16[c // 4][0:kk, c % 4, :], wv_sb[c][0:kk, 0:VH],
                            start=(c == 0), stop=(c == NCH - 1))
                        nc.tensor.matmul(
                            v2_ps[:], hsT16[c // 4][0:kk, c % 4, :], wv_sb[c][0:kk, VH:VW],
                            start=(c == 0), stop=(c == NCH - 1))
                    for c in range(NCH):
                        kk = _chunk_k(c)
                        nc.tensor.matmul(
                            qk_ps[:, 0:QW], hsr[c // 4][0:kk, c % 4, :],
                            wqkl[c][0:kk, 0:QW],
                            start=False, stop=False, skip_group_check=True)
                        nc.tensor.matmul(
                            qk_ps[:, 0:QW], hlo[c // 4][0:kk, c % 4, :],
                            wqkr[c][0:kk, 0:QW],
                            start=False, stop=(c == NCH - 1),
                            skip_group_check=True)

                    # kv for the PREVIOUS tile (its phik/v16 are long ready,
                    # so the PE never blocks on the vector chain below)
                    if t > 0:
                        kv_mm(t - 1)

                    # exact fp32 copies + squares
                    qkf = rot.tile([P, 2 * QW], F32, tag="qkf")
                    nc.scalar.activation(qkf[:], qk_ps[:], ACT)
                    qf32 = qkf[:, 0:QW]
                    kf32 = qkf[:, QW:2 * QW]
                    sq2 = rot.tile([P, QW], F32, tag="sq2")
                    nc.vector.tensor_mul(sq2[:], qk_ps[:, 0:QW], qf32)
                    sk2 = rot.tile([P, QW], F32, tag="sk2")
                    nc.vector.tensor_mul(sk2[:], qk_ps[:, QW:2 * QW], kf32)

                    # ksum-sq accumulator (per-partition partial sums, fp32)
                    if t == 0:
                        nc.vector.tensor_copy(ksq_acc[:], sk2[:])
                    else:
                        nc.vector.tensor_add(ksq_acc[:], ksq_acc[:], sk2[:])

                    # qsum = 1 + sum(q) + 0.5*sum(q^2); rq = mask/qsum
                    sumq = rot.tile([P, HPC], F32, tag="sumq")
                    nc.vector.tensor_reduce(
                        sumq[:], qf32.rearrange("p (h f) -> p h f", f=F),
                        mybir.AxisListType.X, mybir.AluOpType.add)
                    sumq2 = rot.tile([P, HPC], F32, tag="sumq2")
                    nc.vector.tensor_reduce(
                        sumq2[:], sq2[:].rearrange("p (h f) -> p h f", f=F),
                        mybir.AxisListType.X, mybir.AluOpType.add)
                    qsum = rot.tile([P, HPC], F32, tag="qsum")
                    nc.vector.tensor_scalar(
                        qsum[:], sumq2[:], 0.5, 1.0,
                        mybir.AluOpType.mult, mybir.AluOpType.add)
                    nc.vector.tensor_add(qsum[:], qsum[:], sumq[:])
                    rq = rot.tile([P, HPC], F32, tag="rq")
                    nc.vector.reciprocal(rq[:], qsum[:])
                    nc.vector.tensor_mul(
                        rq[:], rq[:], mask_sb[:, t:t + 1].broadcast_to([P, HPC]))
                    rq05 = rot.tile([P, HPC], F32, tag="rq05")
                    nc.vector.tensor_scalar_mul(rq05[:], rq[:], 0.5)

                    # phi_q (rq folded) -> stash (bf16)
                    pq = phiq[:, t]                      # [P, HPC, E]
                    nc.vector.tensor_copy(pq[:, :, 0:1], rq[:].unsqueeze(2))
                    nc.vector.tensor_mul(
                        pq[:, :, 1:1 + F],
                        qf32.rearrange("p (h f) -> p h f", f=F),
                        rq[:].unsqueeze(2).broadcast_to([P, HPC, F]))
                    nc.vector.tensor_mul(
                        pq[:, :, 1 + F:E],
                        sq2[:].rearrange("p (h f) -> p h f", f=F),
                        rq05[:].unsqueeze(2).broadcast_to([P, HPC, F]))

                    # phi_k (bf16, ones col preset) and v (bf16)
                    pk = phik_t[t % 2]
                    nc.scalar.activation(
                        pk[:, :, 1:1 + F],
                        kf32.rearrange("p (h f) -> p h f", f=F), ACT)
                    nc.vector.tensor_scalar_mul(
                        pk[:, :, 1 + F:E],
                        sk2[:].rearrange("p (h f) -> p h f", f=F), 0.5)
                    v16 = v16_t[t % 2]
                    nc.scalar.activation(v16[:, 0:VH], v1_ps[:], ACT)
                    nc.scalar.activation(v16[:, VH:VW], v2_ps[:], ACT)

            kv_mm(NT - 1)

            # =============== MID: ksum assembly (matmul-based) ===============
            with nc.named_scope("mid"):
                # [1, 512] on partition 0: exact (sum_n hs) @ Wk | 0.5*sum(k^2)
                sums_ps = ps2.tile([1, 512], F32, tag="tps", name="sums_ps")
                for c in range(NCH):
                    kk = _chunk_k(c)
                    nc.tensor.matmul(sums_ps[:, 0:QW], hsTsum[0:kk, c:c + 1],
                                     wqk_sb[c][0:kk, QW:2 * QW],
                                     start=(c == 0), stop=(c == NCH - 1),
                                     skip_group_check=True)
                nc.tensor.matmul(sums_ps[:, QW:2 * QW], half_col[:], ksq_acc[:],
                                 start=True, stop=True, skip_group_check=True)

                rk_view = rk_row[:].rearrange("o (h e) -> o h e", e=E)
                nc.vector.memset(rk_view[:, :, 0:1], float(S))
                nc.vector.tensor_copy(
                    rk_view[:, :, 1:1 + F],
                    sums_ps[:, 0:QW].rearrange("o (h f) -> o h f", f=F))
                nc.vector.tensor_copy(
                    rk_view[:, :, 1 + F:E],
                    sums_ps[:, QW:2 * QW].rearrange("o (h f) -> o h f", f=F))
                nc.vector.reciprocal(rk_row[:], rk_row[:])

                # broadcast rk over 65 partitions via PE, then scale kv
                rk_sb = state.tile([E, HPC * E], F32)
                for i in range(2):
                    rk_ps = ps.tile([E, VH], F32, tag=f"v{i + 1}",
                                    name=f"rk_ps{i}")
                    nc.tensor.matmul(rk_ps[:], ones_row[:, 0:E],
                                     rk_row[:, i * VH:(i + 1) * VH],
                                     start=True, stop=True,
                                     skip_group_check=True)
                    nc.scalar.activation(rk_sb[:, i * VH:(i + 1) * VH],
                                         rk_ps[:], ACT)
                    nc.vector.tensor_mul(
                        kvs_sb[:, i * VH:(i + 1) * VH],
                        kv_ps[i][:], rk_sb[:, i * VH:(i + 1) * VH])

            # =============== PASS B (3-deep software pipeline) ===============
            # stages for tile t: T=phiq transpose, O=o matmuls, R=o transpose,
            # W=wo matmuls. body(t) issues T(t+1) O(t) R(t-1) W(t-2).
            phiT_sbs, o_sbs, oT_sbs = {}, {}, {}

            def stage_T(t):                      # phiq -> phiT_sb [E, HPC, P]
                tp = ps2.tile([E, HPC, P], BF16, tag="tps", name=f"ptp_{t}",
                              bufs=2)
                for h in range(HPC):
                    nc.tensor.matmul(tp[:, h, :], phiq[:, t, h, :],
                                     id16_sb[:], is_transpose=True)
                phiT = rot.tile([E, HPC, P], BF16, tag="phiT",
                                name=f"phiT_{t}")
                nc.scalar.activation(
                    phiT[:].rearrange("p h n -> p (h n)"),
                    tp[:].rearrange("p h n -> p (h n)"), ACT)
                phiT_sbs[t] = phiT

            def stage_O(t):                      # o = phiT^T @ kvs [P, VW]
                phiT = phiT_sbs.pop(t)
                o_ps = [ps.tile([P, VH], F32, tag="v1", name=f"ops0_{t}"),
                        ps.tile([P, VH], F32, tag="v2", name=f"ops1_{t}")]
                for h in range(HPC):
                    nc.tensor.matmul(
                        o_ps[h // 4][:, (h % 4) * E:(h % 4) * E + E],
                        phiT[:, h, :], kvs_sb[:, h * E:h * E + E],
                        start=(h % 4 == 0), stop=(h % 4 == 3),
                        skip_group_check=True)
                o_sb = rot.tile([P, VW], BF16, tag="osb", name=f"osb_{t}")
                nc.vector.tensor_copy(o_sb[:, 0:VH], o_ps[0][:])
                nc.scalar.activation(o_sb[:, VH:VW], o_ps[1][:], ACT)
                o_sbs[t] = o_sb

            def stage_R(t):                      # o^T -> oT_sb [P, OCH, P]
                o_sb = o_sbs.pop(t)
                tp = ps.tile([P, OCH, P], BF16, tag="kv1", name=f"otp_{t}")
                for c in range(OCH):
                    kk = OLAST if c == OCH - 1 else P
                    nc.tensor.matmul(tp[0:kk, c, :],
                                     o_sb[:, c * P:c * P + kk],
                                     id16_sb[:], is_transpose=True)
                oT = rot.tile([P, OCH, P], BF16, tag="oT", name=f"oT_{t}")
                nc.vector.tensor_copy(
                    oT[:].rearrange("p c n -> p (c n)"),
                    tp[:].rearrange("p c n -> p (c n)"))
                oT_sbs[t] = oT

            def stage_W(t):                      # out = o^T.T @ Wo -> DMA
                oT = oT_sbs.pop(t)
                f1 = ps2.tile([P, 512], F32, tag="qk", name=f"f1_{t}")
                f2 = ps2.tile([P, 512], F32, tag="qk", name=f"f2_{t}")
                f3 = ps.tile([P, D - 1024], F32, tag="kv0", name=f"f3_{t}")
                for c in range(OCH):
                    kk = OLAST if c == OCH - 1 else P
                    nc.tensor.matmul(f1[:], oT[0:kk, c, :],
                                     wo_sb[0:kk, c, 0:512],
                                     start=(c == 0), stop=(c == OCH - 1))
                    nc.tensor.matmul(f2[:], oT[0:kk, c, :],
                                     wo_sb[0:kk, c, 512:1024],
                                     start=(c == 0), stop=(c == OCH - 1))
                    nc.tensor.matmul(f3[:], oT[0:kk, c, :],
                                     wo_sb[0:kk, c, 1024:D],
                                     start=(c == 0), stop=(c == OCH - 1))
                out_sb = rot.tile([P, D], F32, tag="outsb", name=f"outsb_{t}")
                nc.vector.tensor_copy(out_sb[:, 0:512], f1[:])
                nc.sync.dma_start(out=out[t * P:(t + 1) * P, 0:512],
                                  in_=out_sb[:, 0:512])
                nc.scalar.activation(out_sb[:, 512:1024], f2[:], ACT)
                nc.vector.tensor_copy(out_sb[:, 1024:D], f3[:])
                nc.sync.dma_start(out=out[t * P:(t + 1) * P, 512:D],
                                  in_=out_sb[:, 512:D])

            for b in range(NT + 3):
                with nc.named_scope(f"B{b}"):
                    if b < NT:
                        stage_T(b)
                    if 0 <= b - 1 < NT:
                        stage_O(b - 1)
                    if 0 <= b - 2 < NT:
                        stage_R(b - 2)
                    if 0 <= b - 3 < NT:
                        stage_W(b - 3)

    nc.compile()
    return nc


def _prep_core_inputs(hidden_states, attention_mask, Wq, Wk, Wv, Wo, core):
    b, half = core // 2, core % 2
    h0 = half * HPC
    bf = ml_dtypes.bfloat16

    hs = np.ascontiguousarray(hidden_states[b]).astype(np.float32)
    maskf = np.ascontiguousarray(
        attention_mask[b].astype(np.float32).reshape(NT, P).T)

    def chunks(w):
        out = np.zeros((NCH, P, w.shape[1]), dtype=np.float32)
        for c in range(NCH):
            kk = _chunk_k(c)
            out[c, 0:kk] = w[c * P:c * P + kk]
        return out

    wq_h = Wq[:, h0 * F:(h0 + HPC) * F].astype(np.float32)
    wk_h = Wk[:, h0 * F:(h0 + HPC) * F].astype(np.float32)
    wqk_h = chunks(np.concatenate([wq_h, wk_h], axis=1))
    wv_h = chunks(Wv[:, h0 * E:(h0 + HPC) * E].astype(np.float32)).astype(bf)
    wo_rows = Wo[h0 * E:(h0 + HPC) * E].astype(np.float32)
    wo_h = np.zeros((OCH, P, D), dtype=np.float32)
    for c in range(OCH):
        kk = OLAST if c == OCH - 1 else P
        wo_h[c, 0:kk] = wo_rows[c * P:c * P + kk]
    wo_h = wo_h.astype(bf)

    return {
        "hs": hs,
        "maskf": maskf,
        "wqk": wqk_h,
        "wv": wv_h,
        "wo": wo_h,
        "id32": np.eye(P, dtype=np.float32),
        "id16": np.eye(P, dtype=np.float32).astype(bf),
    }


def kernel(hidden_states, attention_mask, Wq, Wk, Wv, Wo, _trace=False):
    hidden_states = np.asarray(hidden_states)
    attention_mask = np.asarray(attention_mask)
    Wq = np.asarray(Wq); Wk = np.asarray(Wk)
    Wv = np.asarray(Wv); Wo = np.asarray(Wo)

    if "nc" not in _CACHED:
        _CACHED["nc"] = build_bass()
    nc = _CACHED["nc"]

    in_maps = [
        _prep_core_inputs(hidden_states, attention_mask, Wq, Wk, Wv, Wo, c)
        for c in range(8)
    ]
    res = run_bass_kernel_spmd(nc, in_maps, core_ids=list(range(8)),
                               trace=_trace)
    _CACHED["last_result"] = res
    out = np.empty((B, S, D), dtype=np.float32)
    for b in range(B):
        out[b] = res.results[2 * b]["out"] + res.results[2 * b + 1]["out"]
    return out


# revision 35
# speedup vs baseline: 1.0467x; 1.0467x over previous
"""Trainium2 Bass kernel for nn_LinearMultiheadAttention (linear attention with
polynomial feature map phi(x) = [1, x, 0.5 x^2]), sharded over 8 NeuronCores.

Sharding: core c -> batch b = c//2, heads h0 = (c%2)*8 .. h0+8.
Each core computes a partial output (its 8 heads' contribution through Wo);
the host sums the two partials per batch.

Precision: the z = qsum*ksum normalizer is catastrophically ill-conditioned
(qsum = 1 + sum(q) + 0.5 sum(q^2) crosses zero; min |qsum| ~3e-4 while
outputs reach 6e5), so q is computed to full fp32 accuracy via an exact
3-term fp32r split (hs = hi + lo, W = Whi + Wlo, products exact in the fp32
PSUM accumulate; only the lo*lo term is dropped). k needs less: its error
enters through 1/ksumvec, whose sq-slots are chi^2-concentrated (~2048,
never near zero) and whose linear slots (sum_n k, which CAN be near zero)
are recomputed exactly in mid as (sum_n hs) @ Wk in fp32. So k itself is a
single fp32r pass (rel err ~1e-4, plenty for the bf16 phi_k / kv path).
The v / kv / qkv / Wo path is bf16.

Scheduling: kv matmuls deferred one tile (no PE head-of-line block on the
phi_k build), no gpsimd anywhere (measured ~20x below spec), matmul-based
ksum assembly + PE-broadcast of 1/ksum in mid (no SBUF-SBUF DMAs or
transposes), per-chunk weight tiles + early hs prefetch for warmup, pass B
software-pipelined 3 deep (T(b) O(b-1) R(b-2) W(b-3)) with single-bank bf16
transpose targets; all 8 PSUM banks stay allocated across both passes via
tag reuse. Measured ~461 us vs the 770 us predecessor.
"""
import numpy as np
import ml_dtypes

import concourse.tile as tile
from concourse import bacc, mybir
from concourse.bass_utils import run_bass_kernel_spmd

F32 = mybir.dt.float32
F32R = mybir.dt.float32r
BF16 = mybir.dt.bfloat16

B, S, D = 4, 4096, 1040
H, F, E = 16, 32, 65          # heads, feature_dim, head_dim (= 2F+1)
HPC = 8                        # heads per core
P = 128
NT = S // P                    # 32 token tiles per core
NCH = 9                        # ceil(D/128); last chunk K=16
KLAST = D - 8 * P              # 16
QW = HPC * F                   # 256 q (or k) cols per core
VW = HPC * E                   # 520 v cols per core
VH = 4 * E                     # 260
OCH = 5                        # ceil(VW/128); last chunk K=8
OLAST = VW - 4 * P             # 8

_CACHED = {}


def _chunk_k(c):
    return KLAST if c == NCH - 1 else P


def build_bass():
    nc = bacc.Bacc("TRN2", target_bir_lowering=False, debug=False, num_devices=8)
    hst = nc.dram_tensor("hst", [NCH, P, S], F32, kind="ExternalInput").ap()
    maskf = nc.dram_tensor("maskf", [P, NT], F32, kind="ExternalInput").ap()
    wqk = nc.dram_tensor("wqk", [NCH, P, 2 * QW], F32, kind="ExternalInput").ap()
    wv = nc.dram_tensor("wv", [NCH, P, VW], BF16, kind="ExternalInput").ap()
    wo = nc.dram_tensor("wo", [OCH, P, D], BF16, kind="ExternalInput").ap()
    id16 = nc.dram_tensor("id16", [P, P], BF16, kind="ExternalInput").ap()
    out = nc.dram_tensor("out", [S, D], F32, kind="ExternalOutput").ap()

    ACT = mybir.ActivationFunctionType.Copy

    with tile.TileContext(nc) as tc:
        with (
            tc.tile_pool(name="consts", bufs=1) as consts,
            tc.tile_pool(name="state", bufs=1) as state,
            tc.tile_pool(name="rot", bufs=2) as rot,
            tc.tile_pool(name="ps", bufs=1, space="PSUM") as ps,
            tc.tile_pool(name="ps2", bufs=2, space="PSUM") as ps2,
        ):
            # ---- small consts + hs^T prefetch BEFORE the big weight DMAs ----
            hstv = hst.rearrange("c p n -> p c n")
            hs_tiles = [consts.tile([P, NCH, P], F32, name=f"hstt{i}")
                        for i in range(3)]
            nc.sync.dma_start(out=hs_tiles[0], in_=hstv[:, :, 0:P])
            id16_sb = consts.tile([P, P], BF16)
            nc.sync.dma_start(out=id16_sb, in_=id16)
            mask_sb = consts.tile([P, NT], F32)
            nc.sync.dma_start(out=mask_sb, in_=maskf)

            # weights as per-chunk tiles: independent deps, so tile 0's
            # matmuls start as soon as chunk 0's DMA + split land
            wqk_sb = [consts.tile([P, 2 * QW], F32, name=f"wqk{c}")
                      for c in range(NCH)]
            wqkr = [consts.tile([P, 2 * QW], F32R, name=f"wqkr{c}")
                    for c in range(NCH)]
            wqkl = [consts.tile([P, 2 * QW], F32R, name=f"wqkl{c}")
                    for c in range(NCH)]
            wv_sb = [consts.tile([P, VW], BF16, name=f"wv{c}")
                     for c in range(NCH)]
            for c in range(NCH):
                nc.sync.dma_start(out=wqk_sb[c], in_=wqk[c])
                nc.sync.dma_start(out=wv_sb[c], in_=wv[c])
                if c in (1, 2):
                    nc.sync.dma_start(out=hs_tiles[c],
                                      in_=hstv[:, :, c * P:(c + 1) * P])
                nc.vector.tensor_copy(wqkr[c][:], wqk_sb[c][:])
                nc.vector.tensor_tensor(wqkl[c][:], wqk_sb[c][:],
                                        wqkr[c][:].bitcast(F32),
                                        mybir.AluOpType.subtract)
            wo_sb = consts.tile([P, OCH, D], BF16)
            nc.sync.dma_start(out=wo_sb, in_=wo.rearrange("c p j -> p c j"))

            half_col = consts.tile([P, 1], F32)
            nc.vector.memset(half_col[:], 0.5)
            ones_row = consts.tile([1, P], F32)
            nc.vector.memset(ones_row[:], 1.0)

            # ---- persistent state ----
            phiq = state.tile([P, NT, HPC, E], BF16)   # rq-folded phi(q) stash
            kvs_sb = state.tile([E, HPC * E], BF16)    # rksum-scaled kv
            rk_row = state.tile([1, HPC * E], F32)
            # exact per-chunk row-sums of hs^T (for the exact ksum-linear path)
            hsTsum = state.tile([P, NCH], F32)
            ksq_acc = state.tile([P, QW], F32)
            kv_ps = [ps.tile([E, VH], F32, tag=f"kv{i}", name=f"kv{i}")
                     for i in range(2)]
            # phik/v16 live one extra tile (kv deferral) -> explicit 2 bufs
            phik_t = [state.tile([P, HPC, E], BF16, name=f"phik{i}")
                      for i in range(2)]
            v16_t = [state.tile([P, VW], BF16, name=f"v16{i}")
                     for i in range(2)]
            # ones column of phi_k never changes: set once per buffer
            for i in range(2):
                nc.vector.memset(phik_t[i][:, :, 0:1], 1.0)

            def kv_mm(t):
                pk, vv = phik_t[t % 2], v16_t[t % 2]
                for h in range(HPC):
                    nc.tensor.matmul(
                        kv_ps[h // 4][:, (h % 4) * E:(h % 4) * E + E],
                        pk[:, h, :], vv[:, h * E:h * E + E],
                        start=(t == 0 and h % 4 == 0), stop=(t == NT - 1),
                        skip_group_check=True)

            # =============== PASS A ===============
            for t in range(NT):
                with nc.named_scope(f"A{t}"):
                    hs_t = hs_tiles[t % 3]

                    GW = (4, 4, 1)
                    hsr = [rot.tile([P, GW[g], P], F32R, tag=f"hsr{g}",
                                    name=f"hsr{g}_{t}") for g in range(3)]
                    hlo = [rot.tile([P, GW[g], P], F32R, tag=f"hlo{g}",
                                    name=f"hlo{g}_{t}") for g in range(3)]
                    hsT16 = [rot.tile([P, GW[g], P], BF16, tag=f"hsT16{g}",
                                      name=f"hsT16{g}_{t}") for g in range(3)]
                    for g, cs in enumerate([range(0, 4), range(4, 8), range(8, 9)]):
                        lo, hi = cs[0], cs[-1] + 1
                        nch = hi - lo
                        src = hs_t[:, lo:hi, :].rearrange("p c n -> p (c n)")
                        hr = hsr[g][:].rearrange("p c n -> p (c n)")
                        nc.scalar.activation(hr, src, ACT)
                        nc.vector.tensor_tensor(
                            hlo[g][:].rearrange("p c n -> p (c n)"),
                            src, hr.bitcast(F32), mybir.AluOpType.subtract)
                        nc.scalar.activation(
                            hsT16[g][:].rearrange("p c n -> p (c n)"),
                            src, ACT)
                        # exact hs^T row-sums (fp32) for the ksum-linear path
                        red = rot.tile([P, 4], F32, tag="hred", bufs=3,
                                       name=f"red_{t}_{g}")
                        nc.vector.tensor_reduce(
                            red[:, 0:nch], hs_t[:, lo:hi, :],
                            mybir.AxisListType.X, mybir.AluOpType.add)
                        if t == 0:
                            nc.vector.tensor_copy(hsTsum[:, lo:hi],
                                                  red[:, 0:nch])
                        else:
                            nc.vector.tensor_add(hsTsum[:, lo:hi],
                                                 hsTsum[:, lo:hi],
                                                 red[:, 0:nch])

                    # refill this hs slot for tile t+3 (readers above queued)
                    if t + 3 < NT:
                        nc.sync.dma_start(
                            out=hs_t,
                            in_=hstv[:, :, (t + 3) * P:(t + 4) * P])

                    # projections: q|k via exact 3-term fp32r, v via bf16
                    qk_ps = ps2.tile([P, 2 * QW], F32, tag="qk", name=f"qk_{t}")
                    v1_ps = ps.tile([P, VH], F32, tag="v1", name=f"v1_{t}")
                    v2_ps = ps.tile([P, VH], F32, tag="v2", name=f"v2_{t}")
                    # r-pass covers q|k (512); l/lo correction passes cover
                    # only the q half (256): k's error propagates relatively
                    # through 1/ksum (no catastrophic cancellation there).
                    for c in range(NCH):
                        kk = P
                        if c < NCH - 1:
                            nc.tensor.matmul(
                                qk_ps[:], hsr[c // 4][0:kk, c % 4, :], wqkr[c][0:kk, :],
                                start=(c == 0), stop=False,
                                skip_group_check=True)
                        else:
                            # split last chunk so the k region gets its stop
                            nc.tensor.matmul(
                                qk_ps[:, 0:QW], hsr[c // 4][0:kk, c % 4, :],
                                wqkr[c][0:kk, 0:QW],
                                start=False, stop=False,
                                skip_group_check=True)
                            nc.tensor.matmul(
                                qk_ps[:, QW:2 * QW], hsr[c // 4][0:kk, c % 4, :],
                                wqkr[c][0:kk, QW:2 * QW],
                                start=False, stop=True,
                                skip_group_check=True)
                        nc.tensor.matmul(
                            v1_ps[:], hsT16[c // 4][0:kk, c % 4, :], wv_sb[c][0:kk, 0:VH],
                            start=(c == 0), stop=(c == NCH - 1))
                        nc.tensor.matmul(
                            v2_ps[:], hsT16[c // 4][0:kk, c % 4, :], wv_sb[c][0:kk, VH:VW],
                            start=(c == 0), stop=(c == NCH - 1))
                    for c in range(NCH):
                        kk = P
                        nc.tensor.matmul(
                            qk_ps[:, 0:QW], hsr[c // 4][0:kk, c % 4, :],
                            wqkl[c][0:kk, 0:QW],
                            start=False, stop=False, skip_group_check=True)
                        nc.tensor.matmul(
                            qk_ps[:, 0:QW], hlo[c // 4][0:kk, c % 4, :],
                            wqkr[c][0:kk, 0:QW],
                            start=False, stop=(c == NCH - 1),
                            skip_group_check=True)

                    # kv for the PREVIOUS tile (its phik/v16 are long ready,
                    # so the PE never blocks on the vector chain below)
                    if t > 0:
                        kv_mm(t - 1)

                    # exact fp32 copies + squares
                    qkf = rot.tile([P, 2 * QW], F32, tag="qkf")
                    nc.scalar.activation(qkf[:], qk_ps[:], ACT)
                    qf32 = qkf[:, 0:QW]
                    kf32 = qkf[:, QW:2 * QW]
                    sq2 = rot.tile([P, QW], F32, tag="sq2")
                    nc.vector.tensor_mul(sq2[:], qk_ps[:, 0:QW], qf32)
                    sk2 = rot.tile([P, QW], F32, tag="sk2")
                    nc.vector.tensor_mul(sk2[:], qk_ps[:, QW:2 * QW], kf32)

                    # ksum-sq accumulator (per-partition partial sums, fp32)
                    if t == 0:
                        nc.vector.tensor_copy(ksq_acc[:], sk2[:])
                    else:
                        nc.vector.tensor_add(ksq_acc[:], ksq_acc[:], sk2[:])

                    # qsum = 1 + sum(q) + 0.5*sum(q^2); rq = mask/qsum
                    sumq = rot.tile([P, HPC], F32, tag="sumq")
                    nc.vector.tensor_reduce(
                        sumq[:], qf32.rearrange("p (h f) -> p h f", f=F),
                        mybir.AxisListType.X, mybir.AluOpType.add)
                    sumq2 = rot.tile([P, HPC], F32, tag="sumq2")
                    nc.vector.tensor_reduce(
                        sumq2[:], sq2[:].rearrange("p (h f) -> p h f", f=F),
                        mybir.AxisListType.X, mybir.AluOpType.add)
                    qsum = rot.tile([P, HPC], F32, tag="qsum")
                    nc.vector.tensor_scalar(
                        qsum[:], sumq2[:], 0.5, 1.0,
                        mybir.AluOpType.mult, mybir.AluOpType.add)
                    nc.vector.tensor_add(qsum[:], qsum[:], sumq[:])
                    rq = rot.tile([P, HPC], F32, tag="rq")
                    nc.vector.reciprocal(rq[:], qsum[:])
                    nc.vector.tensor_mul(
                        rq[:], rq[:], mask_sb[:, t:t + 1].broadcast_to([P, HPC]))
                    rq05 = rot.tile([P, HPC], F32, tag="rq05")
                    nc.vector.tensor_scalar_mul(rq05[:], rq[:], 0.5)

                    # phi_q (rq folded) -> stash (bf16)
                    pq = phiq[:, t]                      # [P, HPC, E]
                    nc.vector.tensor_copy(pq[:, :, 0:1], rq[:].unsqueeze(2))
                    nc.vector.tensor_mul(
                        pq[:, :, 1:1 + F],
                        qf32.rearrange("p (h f) -> p h f", f=F),
                        rq[:].unsqueeze(2).broadcast_to([P, HPC, F]))
                    nc.vector.tensor_mul(
                        pq[:, :, 1 + F:E],
                        sq2[:].rearrange("p (h f) -> p h f", f=F),
                        rq05[:].unsqueeze(2).broadcast_to([P, HPC, F]))

                    # phi_k (bf16, ones col preset) and v (bf16)
                    pk = phik_t[t % 2]
                    nc.scalar.activation(
                        pk[:, :, 1:1 + F],
                        kf32.rearrange("p (h f) -> p h f", f=F), ACT)
                    nc.vector.tensor_scalar_mul(
                        pk[:, :, 1 + F:E],
                        sk2[:].rearrange("p (h f) -> p h f", f=F), 0.5)
                    v16 = v16_t[t % 2]
                    nc.scalar.activation(v16[:, 0:VH], v1_ps[:], ACT)
                    nc.scalar.activation(v16[:, VH:VW], v2_ps[:], ACT)

            kv_mm(NT - 1)

            # =============== MID: ksum assembly (matmul-based) ===============
            with nc.named_scope("mid"):
                # [1, 512] on partition 0: exact (sum_n hs) @ Wk | 0.5*sum(k^2)
                sums_ps = ps2.tile([1, 512], F32, tag="tps", name="sums_ps")
                for c in range(NCH):
                    kk = P
                    nc.tensor.matmul(sums_ps[:, 0:QW], hsTsum[0:kk, c:c + 1],
                                     wqk_sb[c][0:kk, QW:2 * QW],
                                     start=(c == 0), stop=(c == NCH - 1),
                                     skip_group_check=True)
                nc.tensor.matmul(sums_ps[:, QW:2 * QW], half_col[:], ksq_acc[:],
                                 start=True, stop=True, skip_group_check=True)

                rk_view = rk_row[:].rearrange("o (h e) -> o h e", e=E)
                nc.vector.memset(rk_view[:, :, 0:1], float(S))
                nc.vector.tensor_copy(
                    rk_view[:, :, 1:1 + F],
                    sums_ps[:, 0:QW].rearrange("o (h f) -> o h f", f=F))
                nc.vector.tensor_copy(
                    rk_view[:, :, 1 + F:E],
                    sums_ps[:, QW:2 * QW].rearrange("o (h f) -> o h f", f=F))
                nc.vector.reciprocal(rk_row[:], rk_row[:])

                # broadcast rk over 65 partitions via PE, then scale kv
                rk_sb = state.tile([E, HPC * E], F32)
                for i in range(2):
                    rk_ps = ps.tile([E, VH], F32, tag=f"v{i + 1}",
                                    name=f"rk_ps{i}")
                    nc.tensor.matmul(rk_ps[:], ones_row[:, 0:E],
                                     rk_row[:, i * VH:(i + 1) * VH],
                                     start=True, stop=True,
                                     skip_group_check=True)
                    nc.scalar.activation(rk_sb[:, i * VH:(i + 1) * VH],
                                         rk_ps[:], ACT)
                    nc.vector.tensor_mul(
                        kvs_sb[:, i * VH:(i + 1) * VH],
                        kv_ps[i][:], rk_sb[:, i * VH:(i + 1) * VH])

            # =============== PASS B (3-deep software pipeline) ===============
            # stages for tile t: T=phiq transpose, O=o matmuls, R=o transpose,
            # W=wo matmuls. body(t) issues T(t+1) O(t) R(t-1) W(t-2).
            phiT_sbs, o_sbs, oT_sbs = {}, {}, {}

            def stage_T(t):                      # phiq -> phiT_sb [E, HPC, P]
                tp = ps2.tile([E, HPC, P], BF16, tag="tps", name=f"ptp_{t}",
                              bufs=2)
                for h in range(HPC):
                    nc.tensor.matmul(tp[:, h, :], phiq[:, t, h, :],
                                     id16_sb[:], is_transpose=True)
                phiT = rot.tile([E, HPC, P], BF16, tag="phiT",
                                name=f"phiT_{t}")
                nc.scalar.activation(
                    phiT[:].rearrange("p h n -> p (h n)"),
                    tp[:].rearrange("p h n -> p (h n)"), ACT)
                phiT_sbs[t] = phiT

            def stage_O(t):                      # o = phiT^T @ kvs [P, VW]
                phiT = phiT_sbs.pop(t)
                o_ps = [ps.tile([P, VH], F32, tag="v1", name=f"ops0_{t}"),
                        ps.tile([P, VH], F32, tag="v2", name=f"ops1_{t}")]
                for h in range(HPC):
                    nc.tensor.matmul(
                        o_ps[h // 4][:, (h % 4) * E:(h % 4) * E + E],
                        phiT[:, h, :], kvs_sb[:, h * E:h * E + E],
                        start=(h % 4 == 0), stop=(h % 4 == 3),
                        skip_group_check=True)
                o_sb = rot.tile([P, VW], BF16, tag="osb", name=f"osb_{t}")
                nc.vector.tensor_copy(o_sb[:, 0:VH], o_ps[0][:])
                nc.scalar.activation(o_sb[:, VH:VW], o_ps[1][:], ACT)
                o_sbs[t] = o_sb

            def stage_R(t):                      # o^T -> oT_sb [P, OCH, P]
                o_sb = o_sbs.pop(t)
                tp = ps.tile([P, OCH, P], BF16, tag="kv1", name=f"otp_{t}")
                for c in range(OCH):
                    kk = OLAST if c == OCH - 1 else P
                    nc.tensor.matmul(tp[0:kk, c, :],
                                     o_sb[:, c * P:c * P + kk],
                                     id16_sb[:], is_transpose=True)
                oT = rot.tile([P, OCH, P], BF16, tag="oT", name=f"oT_{t}")
                nc.vector.tensor_copy(
                    oT[:].rearrange("p c n -> p (c n)"),
                    tp[:].rearrange("p c n -> p (c n)"))
                oT_sbs[t] = oT

            def stage_W(t):                      # out = o^T.T @ Wo -> DMA
                oT = oT_sbs.pop(t)
                f1 = ps2.tile([P, 512], F32, tag="qk", name=f"f1_{t}")
                f2 = ps2.tile([P, 512], F32, tag="qk", name=f"f2_{t}")
                f3 = ps.tile([P, D - 1024], F32, tag="kv0", name=f"f3_{t}")
                for c in range(OCH):
                    kk = OLAST if c == OCH - 1 else P
                    nc.tensor.matmul(f1[:], oT[0:kk, c, :],
                                     wo_sb[0:kk, c, 0:512],
                                     start=(c == 0), stop=(c == OCH - 1))
                    nc.tensor.matmul(f2[:], oT[0:kk, c, :],
                                     wo_sb[0:kk, c, 512:1024],
                                     start=(c == 0), stop=(c == OCH - 1))
                    nc.tensor.matmul(f3[:], oT[0:kk, c, :],
                                     wo_sb[0:kk, c, 1024:D],
                                     start=(c == 0), stop=(c == OCH - 1))
                out_sb = rot.tile([P, D], F32, tag="outsb", name=f"outsb_{t}")
                nc.vector.tensor_copy(out_sb[:, 0:512], f1[:])
                nc.sync.dma_start(out=out[t * P:(t + 1) * P, 0:512],
                                  in_=out_sb[:, 0:512])
                nc.scalar.activation(out_sb[:, 512:1024], f2[:], ACT)
                nc.vector.tensor_copy(out_sb[:, 1024:D], f3[:])
                nc.sync.dma_start(out=out[t * P:(t + 1) * P, 512:D],
                                  in_=out_sb[:, 512:D])

            for b in range(NT + 3):
                with nc.named_scope(f"B{b}"):
                    if b < NT:
                        stage_T(b)
                    if 0 <= b - 1 < NT:
                        stage_O(b - 1)
                    if 0 <= b - 2 < NT:
                        stage_R(b - 2)
                    if 0 <= b - 3 < NT:
                        stage_W(b - 3)

    nc.compile()
    return nc


def _prep_core_inputs(hidden_states, attention_mask, Wq, Wk, Wv, Wo, core):
    b, half = core // 2, core % 2
    h0 = half * HPC
    bf = ml_dtypes.bfloat16

    hsT = hidden_states[b].astype(np.float32).T   # [D, S]
    hst = np.zeros((NCH, P, S), dtype=np.float32)
    for c in range(NCH):
        kk = _chunk_k(c)
        hst[c, 0:kk] = hsT[c * P:c * P + kk]
    maskf = np.ascontiguousarray(
        attention_mask[b].astype(np.float32).reshape(NT, P).T)

    def chunks(w):
        out = np.zeros((NCH, P, w.shape[1]), dtype=np.float32)
        for c in range(NCH):
            kk = _chunk_k(c)
            out[c, 0:kk] = w[c * P:c * P + kk]
        return out

    wq_h = Wq[:, h0 * F:(h0 + HPC) * F].astype(np.float32)
    wk_h = Wk[:, h0 * F:(h0 + HPC) * F].astype(np.float32)
    wqk_h = chunks(np.concatenate([wq_h, wk_h], axis=1))
    wv_h = chunks(Wv[:, h0 * E:(h0 + HPC) * E].astype(np.float32)).astype(bf)
    wo_rows = Wo[h0 * E:(h0 + HPC) * E].astype(np.float32)
    wo_h = np.zeros((OCH, P, D), dtype=np.float32)
    for c in range(OCH):
        kk = OLAST if c == OCH - 1 else P
        wo_h[c, 0:kk] = wo_rows[c * P:c * P + kk]
    wo_h = wo_h.astype(bf)

    return {
        "hst": hst,
        "maskf": maskf,
        "wqk": wqk_h,
        "wv": wv_h,
        "wo": wo_h,
        "id16": np.eye(P, dtype=np.float32).astype(bf),
    }


def kernel(hidden_states, attention_mask, Wq, Wk, Wv, Wo, _trace=False):
    hidden_states = np.asarray(hidden_states)
    attention_mask = np.asarray(attention_mask)
    Wq = np.asarray(Wq); Wk = np.asarray(Wk)
    Wv = np.asarray(Wv); Wo = np.asarray(Wo)

    if "nc" not in _CACHED:
        _CACHED["nc"] = build_bass()
    nc = _CACHED["nc"]

    in_maps = [
        _prep_core_inputs(hidden_states, attention_mask, Wq, Wk, Wv, Wo, c)
        for c in range(8)
    ]
    res = run_bass_kernel_spmd(nc, in_maps, core_ids=list(range(8)),
                               trace=_trace)
    _CACHED["last_result"] = res
    out = np.empty((B, S, D), dtype=np.float32)
    for b in range(B):
        out[b] = res.results[2 * b]["out"] + res.results[2 * b + 1]["out"]
    return out


# revision 36
# speedup vs baseline: 1.2427x; 1.1872x over previous
"""Trainium2 Bass kernel for nn_LinearMultiheadAttention (linear attention with
polynomial feature map phi(x) = [1, x, 0.5 x^2]), sharded over 8 NeuronCores.

Sharding: core c -> batch b = c//2, heads h0 = (c%2)*8 .. h0+8.
Each core computes a partial output (its 8 heads' contribution through Wo);
the host sums the two partials per batch.

Precision: the z = qsum*ksum normalizer is catastrophically ill-conditioned
(qsum = 1 + sum(q) + 0.5 sum(q^2) crosses zero; min |qsum| ~3e-4 while
outputs reach 6e5), so q is computed to full fp32 accuracy via an exact
3-term fp32r split (hs = hi + lo, W = Whi + Wlo, products exact in the fp32
PSUM accumulate; only the lo*lo term is dropped). k needs less: its error
enters through 1/ksumvec, whose sq-slots are chi^2-concentrated (~2048,
never near zero) and whose linear slots (sum_n k, which CAN be near zero)
are recomputed exactly in mid as (sum_n hs) @ Wk in fp32. So k itself is a
single fp32r pass (rel err ~1e-4, plenty for the bf16 phi_k / kv path).
The v / kv / qkv / Wo path is bf16.

Scheduling: kv matmuls deferred one tile (no PE head-of-line block on the
phi_k build), no gpsimd anywhere (measured ~20x below spec), matmul-based
ksum assembly + PE-broadcast of 1/ksum in mid (no SBUF-SBUF DMAs or
transposes), per-chunk weight tiles + early hs prefetch for warmup, pass B
software-pipelined 3 deep (T(b) O(b-1) R(b-2) W(b-3)) with single-bank bf16
transpose targets; all 8 PSUM banks stay allocated across both passes via
tag reuse. Measured ~461 us vs the 770 us predecessor.
"""
import numpy as np
import ml_dtypes

import concourse.tile as tile
from concourse import bacc, mybir
from concourse.bass_utils import run_bass_kernel_spmd

F32 = mybir.dt.float32
F32R = mybir.dt.float32r
BF16 = mybir.dt.bfloat16

B, S, D = 4, 4096, 1040
H, F, E = 16, 32, 65          # heads, feature_dim, head_dim (= 2F+1)
HPC = 8                        # heads per core
P = 128
NT = S // P                    # 32 token tiles per core
NCH = 9                        # ceil(D/128); last chunk K=16
KLAST = D - 8 * P              # 16
QW = HPC * F                   # 256 q (or k) cols per core
VW = HPC * E                   # 520 v cols per core
VH = 4 * E                     # 260
OCH = 5                        # ceil(VW/128); last chunk K=8
OLAST = VW - 4 * P             # 8

_CACHED = {}


def _chunk_k(c):
    return KLAST if c == NCH - 1 else P


def build_bass():
    nc = bacc.Bacc("TRN2", target_bir_lowering=False, debug=False, num_devices=8)
    hst = nc.dram_tensor("hst", [NCH, P, S], F32, kind="ExternalInput").ap()
    maskf = nc.dram_tensor("maskf", [P, NT], F32, kind="ExternalInput").ap()
    wqk = nc.dram_tensor("wqk", [NCH, P, 2 * QW], F32, kind="ExternalInput").ap()
    wv = nc.dram_tensor("wv", [NCH, P, VW], BF16, kind="ExternalInput").ap()
    wo = nc.dram_tensor("wo", [OCH, P, D], BF16, kind="ExternalInput").ap()
    id16 = nc.dram_tensor("id16", [P, P], BF16, kind="ExternalInput").ap()
    out = nc.dram_tensor("out", [S, D], F32, kind="ExternalOutput").ap()

    ACT = mybir.ActivationFunctionType.Copy

    with tile.TileContext(nc) as tc:
        with (
            tc.tile_pool(name="consts", bufs=1) as consts,
            tc.tile_pool(name="state", bufs=1) as state,
            tc.tile_pool(name="rot", bufs=2) as rot,
            tc.tile_pool(name="ps", bufs=1, space="PSUM") as ps,
            tc.tile_pool(name="ps2", bufs=2, space="PSUM") as ps2,
        ):
            # ---- small consts + hs^T prefetch BEFORE the big weight DMAs ----
            hstv = hst.rearrange("c p n -> p c n")
            hs_tiles = [consts.tile([P, NCH, P], F32, name=f"hstt{i}")
                        for i in range(3)]
            nc.sync.dma_start(out=hs_tiles[0], in_=hstv[:, :, 0:P])
            id16_sb = consts.tile([P, P], BF16)
            nc.sync.dma_start(out=id16_sb, in_=id16)
            mask_sb = consts.tile([P, NT], F32)
            nc.sync.dma_start(out=mask_sb, in_=maskf)

            # weights as per-chunk tiles: independent deps, so tile 0's
            # matmuls start as soon as chunk 0's DMA + split land
            wqk_sb = [consts.tile([P, 2 * QW], F32, name=f"wqk{c}")
                      for c in range(NCH)]
            wqkr = [consts.tile([P, 2 * QW], F32R, name=f"wqkr{c}")
                    for c in range(NCH)]
            wqkl = [consts.tile([P, 2 * QW], F32R, name=f"wqkl{c}")
                    for c in range(NCH)]
            wv_sb = [consts.tile([P, VW], BF16, name=f"wv{c}")
                     for c in range(NCH)]
            for c in range(NCH):
                nc.sync.dma_start(out=wqk_sb[c], in_=wqk[c])
                if c == 0:
                    nc.sync.dma_start(out=wv_sb[0], in_=wv[0])
                nc.vector.tensor_copy(wqkr[c][:], wqk_sb[c][:])
                nc.vector.tensor_tensor(wqkl[c][:], wqk_sb[c][:],
                                        wqkr[c][:].bitcast(F32),
                                        mybir.AluOpType.subtract)
            for c in range(1, NCH):
                nc.sync.dma_start(out=wv_sb[c], in_=wv[c])
            for i in (1, 2):
                nc.sync.dma_start(out=hs_tiles[i],
                                  in_=hstv[:, :, i * P:(i + 1) * P])
            wo_sb = consts.tile([P, OCH, D], BF16)
            nc.sync.dma_start(out=wo_sb, in_=wo.rearrange("c p j -> p c j"))

            half_col = consts.tile([P, 1], F32)
            nc.vector.memset(half_col[:], 0.5)
            ones_row = consts.tile([1, P], F32)
            nc.vector.memset(ones_row[:], 1.0)

            # ---- persistent state ----
            phiq = state.tile([P, NT, HPC, E], BF16)   # rq-folded phi(q) stash
            kvs_sb = state.tile([E, HPC * E], BF16)    # rksum-scaled kv
            rk_row = state.tile([1, HPC * E], F32)
            # exact per-chunk row-sums of hs^T (for the exact ksum-linear path)
            hsTsum = state.tile([P, NCH], F32)
            ksq_acc = state.tile([P, QW], F32)
            kv_ps = [ps.tile([E, VH], F32, tag=f"kv{i}", name=f"kv{i}")
                     for i in range(2)]
            # phik/v16 live one extra tile (kv deferral) -> explicit 2 bufs
            phik_t = [state.tile([P, HPC, E], BF16, name=f"phik{i}")
                      for i in range(2)]
            v16_t = [state.tile([P, VW], BF16, name=f"v16{i}")
                     for i in range(2)]
            # ones column of phi_k never changes: set once per buffer
            for i in range(2):
                nc.vector.memset(phik_t[i][:, :, 0:1], 1.0)

            def kv_mm(t):
                pk, vv = phik_t[t % 2], v16_t[t % 2]
                for h in range(HPC):
                    nc.tensor.matmul(
                        kv_ps[h // 4][:, (h % 4) * E:(h % 4) * E + E],
                        pk[:, h, :], vv[:, h * E:h * E + E],
                        start=(t == 0 and h % 4 == 0), stop=(t == NT - 1),
                        skip_group_check=True)

            # =============== PASS A ===============
            for t in range(NT):
                with nc.named_scope(f"A{t}"):
                    hs_t = hs_tiles[t % 3]

                    GW = (4, 4, 1)
                    hsr = [rot.tile([P, GW[g], P], F32R, tag=f"hsr{g}",
                                    name=f"hsr{g}_{t}") for g in range(3)]
                    hlo = [rot.tile([P, GW[g], P], F32R, tag=f"hlo{g}",
                                    name=f"hlo{g}_{t}") for g in range(3)]
                    hsT16 = [rot.tile([P, GW[g], P], BF16, tag=f"hsT16{g}",
                                      name=f"hsT16{g}_{t}") for g in range(3)]
                    for g, cs in enumerate([range(0, 4), range(4, 8), range(8, 9)]):
                        lo, hi = cs[0], cs[-1] + 1
                        nch = hi - lo
                        src = hs_t[:, lo:hi, :].rearrange("p c n -> p (c n)")
                        hr = hsr[g][:].rearrange("p c n -> p (c n)")
                        nc.scalar.activation(hr, src, ACT)
                        nc.vector.tensor_tensor(
                            hlo[g][:].rearrange("p c n -> p (c n)"),
                            src, hr.bitcast(F32), mybir.AluOpType.subtract)
                        nc.scalar.activation(
                            hsT16[g][:].rearrange("p c n -> p (c n)"),
                            src, ACT)
                        # exact hs^T row-sums (fp32) for the ksum-linear path
                        red = rot.tile([P, 4], F32, tag="hred", bufs=3,
                                       name=f"red_{t}_{g}")
                        nc.vector.tensor_reduce(
                            red[:, 0:nch], hs_t[:, lo:hi, :],
                            mybir.AxisListType.X, mybir.AluOpType.add)
                        if t == 0:
                            nc.vector.tensor_copy(hsTsum[:, lo:hi],
                                                  red[:, 0:nch])
                        else:
                            nc.vector.tensor_add(hsTsum[:, lo:hi],
                                                 hsTsum[:, lo:hi],
                                                 red[:, 0:nch])

                    # refill this hs slot for tile t+3 (readers above queued)
                    if t + 3 < NT:
                        nc.sync.dma_start(
                            out=hs_t,
                            in_=hstv[:, :, (t + 3) * P:(t + 4) * P])

                    # projections: q|k via exact 3-term fp32r, v via bf16
                    qk_ps = ps2.tile([P, 2 * QW], F32, tag="qk", name=f"qk_{t}")
                    v1_ps = ps.tile([P, VH], F32, tag="v1", name=f"v1_{t}")
                    v2_ps = ps.tile([P, VH], F32, tag="v2", name=f"v2_{t}")
                    # r-pass covers q|k (512); l/lo correction passes cover
                    # only the q half (256): k's error propagates relatively
                    # through 1/ksum (no catastrophic cancellation there).
                    for c in range(NCH):
                        kk = P
                        if c < NCH - 1:
                            nc.tensor.matmul(
                                qk_ps[:], hsr[c // 4][0:kk, c % 4, :], wqkr[c][0:kk, :],
                                start=(c == 0), stop=False,
                                skip_group_check=True)
                        else:
                            # split last chunk so the k region gets its stop
                            nc.tensor.matmul(
                                qk_ps[:, 0:QW], hsr[c // 4][0:kk, c % 4, :],
                                wqkr[c][0:kk, 0:QW],
                                start=False, stop=False,
                                skip_group_check=True)
                            nc.tensor.matmul(
                                qk_ps[:, QW:2 * QW], hsr[c // 4][0:kk, c % 4, :],
                                wqkr[c][0:kk, QW:2 * QW],
                                start=False, stop=True,
                                skip_group_check=True)
                        nc.tensor.matmul(
                            v1_ps[:], hsT16[c // 4][0:kk, c % 4, :], wv_sb[c][0:kk, 0:VH],
                            start=(c == 0), stop=(c == NCH - 1))
                        nc.tensor.matmul(
                            v2_ps[:], hsT16[c // 4][0:kk, c % 4, :], wv_sb[c][0:kk, VH:VW],
                            start=(c == 0), stop=(c == NCH - 1))
                    for c in range(NCH):
                        kk = P
                        nc.tensor.matmul(
                            qk_ps[:, 0:QW], hsr[c // 4][0:kk, c % 4, :],
                            wqkl[c][0:kk, 0:QW],
                            start=False, stop=False, skip_group_check=True)
                        nc.tensor.matmul(
                            qk_ps[:, 0:QW], hlo[c // 4][0:kk, c % 4, :],
                            wqkr[c][0:kk, 0:QW],
                            start=False, stop=(c == NCH - 1),
                            skip_group_check=True)

                    # kv for the PREVIOUS tile (its phik/v16 are long ready,
                    # so the PE never blocks on the vector chain below)
                    if t > 0:
                        kv_mm(t - 1)

                    # exact fp32 copies + squares
                    qkf = rot.tile([P, 2 * QW], F32, tag="qkf")
                    nc.scalar.activation(qkf[:], qk_ps[:], ACT)
                    qf32 = qkf[:, 0:QW]
                    kf32 = qkf[:, QW:2 * QW]
                    sq2 = rot.tile([P, QW], F32, tag="sq2")
                    nc.vector.tensor_mul(sq2[:], qk_ps[:, 0:QW], qf32)
                    sk2 = rot.tile([P, QW], F32, tag="sk2")
                    nc.vector.tensor_mul(sk2[:], qk_ps[:, QW:2 * QW], kf32)

                    # ksum-sq accumulator (per-partition partial sums, fp32)
                    if t == 0:
                        nc.vector.tensor_copy(ksq_acc[:], sk2[:])
                    else:
                        nc.vector.tensor_add(ksq_acc[:], ksq_acc[:], sk2[:])

                    # qsum = 1 + sum(q) + 0.5*sum(q^2); rq = mask/qsum
                    sumq = rot.tile([P, HPC], F32, tag="sumq")
                    nc.vector.tensor_reduce(
                        sumq[:], qf32.rearrange("p (h f) -> p h f", f=F),
                        mybir.AxisListType.X, mybir.AluOpType.add)
                    sumq2 = rot.tile([P, HPC], F32, tag="sumq2")
                    nc.vector.tensor_reduce(
                        sumq2[:], sq2[:].rearrange("p (h f) -> p h f", f=F),
                        mybir.AxisListType.X, mybir.AluOpType.add)
                    qsum = rot.tile([P, HPC], F32, tag="qsum")
                    nc.vector.tensor_scalar(
                        qsum[:], sumq2[:], 0.5, 1.0,
                        mybir.AluOpType.mult, mybir.AluOpType.add)
                    nc.vector.tensor_add(qsum[:], qsum[:], sumq[:])
                    rq = rot.tile([P, HPC], F32, tag="rq")
                    nc.vector.reciprocal(rq[:], qsum[:])
                    nc.vector.tensor_mul(
                        rq[:], rq[:], mask_sb[:, t:t + 1].broadcast_to([P, HPC]))
                    rq05 = rot.tile([P, HPC], F32, tag="rq05")
                    nc.vector.tensor_scalar_mul(rq05[:], rq[:], 0.5)

                    # phi_q (rq folded) -> stash (bf16)
                    pq = phiq[:, t]                      # [P, HPC, E]
                    nc.vector.tensor_copy(pq[:, :, 0:1], rq[:].unsqueeze(2))
                    nc.vector.tensor_mul(
                        pq[:, :, 1:1 + F],
                        qf32.rearrange("p (h f) -> p h f", f=F),
                        rq[:].unsqueeze(2).broadcast_to([P, HPC, F]))
                    nc.vector.tensor_mul(
                        pq[:, :, 1 + F:E],
                        sq2[:].rearrange("p (h f) -> p h f", f=F),
                        rq05[:].unsqueeze(2).broadcast_to([P, HPC, F]))

                    # phi_k (bf16, ones col preset) and v (bf16)
                    pk = phik_t[t % 2]
                    nc.scalar.activation(
                        pk[:, :, 1:1 + F],
                        kf32.rearrange("p (h f) -> p h f", f=F), ACT)
                    nc.vector.tensor_scalar_mul(
                        pk[:, :, 1 + F:E],
                        sk2[:].rearrange("p (h f) -> p h f", f=F), 0.5)
                    v16 = v16_t[t % 2]
                    nc.scalar.activation(v16[:, 0:VH], v1_ps[:], ACT)
                    nc.scalar.activation(v16[:, VH:VW], v2_ps[:], ACT)

            kv_mm(NT - 1)

            # =============== MID: ksum assembly (matmul-based) ===============
            with nc.named_scope("mid"):
                # [1, 512] on partition 0: exact (sum_n hs) @ Wk | 0.5*sum(k^2)
                sums_ps = ps2.tile([1, 512], F32, tag="tps", name="sums_ps")
                for c in range(NCH):
                    kk = P
                    nc.tensor.matmul(sums_ps[:, 0:QW], hsTsum[0:kk, c:c + 1],
                                     wqk_sb[c][0:kk, QW:2 * QW],
                                     start=(c == 0), stop=(c == NCH - 1),
                                     skip_group_check=True)
                nc.tensor.matmul(sums_ps[:, QW:2 * QW], half_col[:], ksq_acc[:],
                                 start=True, stop=True, skip_group_check=True)

                rk_view = rk_row[:].rearrange("o (h e) -> o h e", e=E)
                nc.vector.memset(rk_view[:, :, 0:1], float(S))
                nc.vector.tensor_copy(
                    rk_view[:, :, 1:1 + F],
                    sums_ps[:, 0:QW].rearrange("o (h f) -> o h f", f=F))
                nc.vector.tensor_copy(
                    rk_view[:, :, 1 + F:E],
                    sums_ps[:, QW:2 * QW].rearrange("o (h f) -> o h f", f=F))
                nc.vector.reciprocal(rk_row[:], rk_row[:])

                # broadcast rk over 65 partitions via PE, then scale kv
                rk_sb = state.tile([E, HPC * E], F32)
                for i in range(2):
                    rk_ps = ps.tile([E, VH], F32, tag=f"v{i + 1}",
                                    name=f"rk_ps{i}")
                    nc.tensor.matmul(rk_ps[:], ones_row[:, 0:E],
                                     rk_row[:, i * VH:(i + 1) * VH],
                                     start=True, stop=True,
                                     skip_group_check=True)
                    nc.scalar.activation(rk_sb[:, i * VH:(i + 1) * VH],
                                         rk_ps[:], ACT)
                    nc.vector.tensor_mul(
                        kvs_sb[:, i * VH:(i + 1) * VH],
                        kv_ps[i][:], rk_sb[:, i * VH:(i + 1) * VH])

            # =============== PASS B (3-deep software pipeline) ===============
            # stages for tile t: T=phiq transpose, O=o matmuls, R=o transpose,
            # W=wo matmuls. body(t) issues T(t+1) O(t) R(t-1) W(t-2).
            phiT_sbs, o_sbs, oT_sbs = {}, {}, {}

            def stage_T(t):                      # phiq -> phiT_sb [E, HPC, P]
                tp = ps2.tile([E, HPC, P], BF16, tag="tps", name=f"ptp_{t}",
                              bufs=2)
                for h in range(HPC):
                    nc.tensor.matmul(tp[:, h, :], phiq[:, t, h, :],
                                     id16_sb[:], is_transpose=True)
                phiT = rot.tile([E, HPC, P], BF16, tag="phiT",
                                name=f"phiT_{t}")
                nc.scalar.activation(
                    phiT[:].rearrange("p h n -> p (h n)"),
                    tp[:].rearrange("p h n -> p (h n)"), ACT)
                phiT_sbs[t] = phiT

            def stage_O(t):                      # o = phiT^T @ kvs [P, VW]
                phiT = phiT_sbs.pop(t)
                o_ps = [ps.tile([P, VH], F32, tag="v1", name=f"ops0_{t}"),
                        ps.tile([P, VH], F32, tag="v2", name=f"ops1_{t}")]
                for h in range(HPC):
                    nc.tensor.matmul(
                        o_ps[h // 4][:, (h % 4) * E:(h % 4) * E + E],
                        phiT[:, h, :], kvs_sb[:, h * E:h * E + E],
                        start=(h % 4 == 0), stop=(h % 4 == 3),
                        skip_group_check=True)
                o_sb = rot.tile([P, VW], BF16, tag="osb", name=f"osb_{t}")
                nc.vector.tensor_copy(o_sb[:, 0:VH], o_ps[0][:])
                nc.scalar.activation(o_sb[:, VH:VW], o_ps[1][:], ACT)
                o_sbs[t] = o_sb

            def stage_R(t):                      # o^T -> oT_sb [P, OCH, P]
                o_sb = o_sbs.pop(t)
                tp = ps.tile([P, OCH, P], BF16, tag="kv1", name=f"otp_{t}")
                for c in range(OCH):
                    kk = OLAST if c == OCH - 1 else P
                    nc.tensor.matmul(tp[0:kk, c, :],
                                     o_sb[:, c * P:c * P + kk],
                                     id16_sb[:], is_transpose=True)
                oT = rot.tile([P, OCH, P], BF16, tag="oT", name=f"oT_{t}")
                nc.vector.tensor_copy(
                    oT[:].rearrange("p c n -> p (c n)"),
                    tp[:].rearrange("p c n -> p (c n)"))
                oT_sbs[t] = oT

            def stage_W(t):                      # out = o^T.T @ Wo -> DMA
                oT = oT_sbs.pop(t)
                f1 = ps2.tile([P, 512], F32, tag="qk", name=f"f1_{t}")
                f2 = ps2.tile([P, 512], F32, tag="qk", name=f"f2_{t}")
                f3 = ps.tile([P, D - 1024], F32, tag="kv0", name=f"f3_{t}")
                for c in range(OCH):
                    kk = OLAST if c == OCH - 1 else P
                    nc.tensor.matmul(f1[:], oT[0:kk, c, :],
                                     wo_sb[0:kk, c, 0:512],
                                     start=(c == 0), stop=(c == OCH - 1))
                    nc.tensor.matmul(f2[:], oT[0:kk, c, :],
                                     wo_sb[0:kk, c, 512:1024],
                                     start=(c == 0), stop=(c == OCH - 1))
                    nc.tensor.matmul(f3[:], oT[0:kk, c, :],
                                     wo_sb[0:kk, c, 1024:D],
                                     start=(c == 0), stop=(c == OCH - 1))
                out_sb = rot.tile([P, D], F32, tag="outsb", name=f"outsb_{t}")
                nc.vector.tensor_copy(out_sb[:, 0:512], f1[:])
                nc.sync.dma_start(out=out[t * P:(t + 1) * P, 0:512],
                                  in_=out_sb[:, 0:512])
                nc.scalar.activation(out_sb[:, 512:1024], f2[:], ACT)
                nc.vector.tensor_copy(out_sb[:, 1024:D], f3[:])
                nc.sync.dma_start(out=out[t * P:(t + 1) * P, 512:D],
                                  in_=out_sb[:, 512:D])

            for b in range(NT + 3):
                with nc.named_scope(f"B{b}"):
                    if b < NT:
                        stage_T(b)
                    if 0 <= b - 1 < NT:
                        stage_O(b - 1)
                    if 0 <= b - 2 < NT:
                        stage_R(b - 2)
                    if 0 <= b - 3 < NT:
                        stage_W(b - 3)

    nc.compile()
    return nc


def _prep_core_inputs(hidden_states, attention_mask, Wq, Wk, Wv, Wo, core):
    b, half = core // 2, core % 2
    h0 = half * HPC
    bf = ml_dtypes.bfloat16

    hsT = hidden_states[b].astype(np.float32).T   # [D, S]
    hst = np.zeros((NCH, P, S), dtype=np.float32)
    for c in range(NCH):
        kk = _chunk_k(c)
        hst[c, 0:kk] = hsT[c * P:c * P + kk]
    maskf = np.ascontiguousarray(
        attention_mask[b].astype(np.float32).reshape(NT, P).T)

    def chunks(w):
        out = np.zeros((NCH, P, w.shape[1]), dtype=np.float32)
        for c in range(NCH):
            kk = _chunk_k(c)
            out[c, 0:kk] = w[c * P:c * P + kk]
        return out

    wq_h = Wq[:, h0 * F:(h0 + HPC) * F].astype(np.float32)
    wk_h = Wk[:, h0 * F:(h0 + HPC) * F].astype(np.float32)
    wqk_h = chunks(np.concatenate([wq_h, wk_h], axis=1))
    wv_h = chunks(Wv[:, h0 * E:(h0 + HPC) * E].astype(np.float32)).astype(bf)
    wo_rows = Wo[h0 * E:(h0 + HPC) * E].astype(np.float32)
    wo_h = np.zeros((OCH, P, D), dtype=np.float32)
    for c in range(OCH):
        kk = OLAST if c == OCH - 1 else P
        wo_h[c, 0:kk] = wo_rows[c * P:c * P + kk]
    wo_h = wo_h.astype(bf)

    return {
        "hst": hst,
        "maskf": maskf,
        "wqk": wqk_h,
        "wv": wv_h,
        "wo": wo_h,
        "id16": np.eye(P, dtype=np.float32).astype(bf),
    }


def kernel(hidden_states, attention_mask, Wq, Wk, Wv, Wo, _trace=False):
    hidden_states = np.asarray(hidden_states)
    attention_mask = np.asarray(attention_mask)
    Wq = np.asarray(Wq); Wk = np.asarray(Wk)
    Wv = np.asarray(Wv); Wo = np.asarray(Wo)

    if "nc" not in _CACHED:
        _CACHED["nc"] = build_bass()
    nc = _CACHED["nc"]

    in_maps = [
        _prep_core_inputs(hidden_states, attention_mask, Wq, Wk, Wv, Wo, c)
        for c in range(8)
    ]
    res = run_bass_kernel_spmd(nc, in_maps, core_ids=list(range(8)),
                               trace=_trace)
    _CACHED["last_result"] = res
    out = np.empty((B, S, D), dtype=np.float32)
    for b in range(B):
        out[b] = res.results[2 * b]["out"] + res.results[2 * b + 1]["out"]
    return out


# revision 37
# speedup vs baseline: 1.2465x; 1.0031x over previous
"""Trainium2 Bass kernel for nn_LinearMultiheadAttention (linear attention with
polynomial feature map phi(x) = [1, x, 0.5 x^2]), sharded over 8 NeuronCores.

Sharding: core c -> batch b = c//2, heads h0 = (c%2)*8 .. h0+8.
Each core computes a partial output (its 8 heads' contribution through Wo);
the host sums the two partials per batch.

Precision: the z = qsum*ksum normalizer is catastrophically ill-conditioned
(qsum = 1 + sum(q) + 0.5 sum(q^2) crosses zero; min |qsum| ~3e-4 while
outputs reach 6e5), so q is computed to full fp32 accuracy via an exact
3-term fp32r split (hs = hi + lo, W = Whi + Wlo, products exact in the fp32
PSUM accumulate; only the lo*lo term is dropped). k needs less: its error
enters through 1/ksumvec, whose sq-slots are chi^2-concentrated (~2048,
never near zero) and whose linear slots (sum_n k, which CAN be near zero)
are recomputed exactly in mid as (sum_n hs) @ Wk in fp32. So k itself is a
single fp32r pass (rel err ~1e-4, plenty for the bf16 phi_k / kv path).
The v / kv / qkv / Wo path is bf16.

Scheduling: kv matmuls deferred one tile (no PE head-of-line block on the
phi_k build), no gpsimd anywhere (measured ~20x below spec), matmul-based
ksum assembly + PE-broadcast of 1/ksum in mid (no SBUF-SBUF DMAs or
transposes), per-chunk weight tiles + early hs prefetch for warmup, pass B
software-pipelined 3 deep (T(b) O(b-1) R(b-2) W(b-3)) with single-bank bf16
transpose targets; all 8 PSUM banks stay allocated across both passes via
tag reuse. hs is shipped pre-transposed/pre-chunked from the host, so
pass A has no PE transposes at all. Measures 370-445 us (device clock
state varies run-to-run) vs the 770-790 us predecessor.
"""
import numpy as np
import ml_dtypes

import concourse.tile as tile
from concourse import bacc, mybir
from concourse.bass_utils import run_bass_kernel_spmd

F32 = mybir.dt.float32
F32R = mybir.dt.float32r
BF16 = mybir.dt.bfloat16

B, S, D = 4, 4096, 1040
H, F, E = 16, 32, 65          # heads, feature_dim, head_dim (= 2F+1)
HPC = 8                        # heads per core
P = 128
NT = S // P                    # 32 token tiles per core
NCH = 9                        # ceil(D/128); last chunk K=16
KLAST = D - 8 * P              # 16
QW = HPC * F                   # 256 q (or k) cols per core
VW = HPC * E                   # 520 v cols per core
VH = 4 * E                     # 260
OCH = 5                        # ceil(VW/128); last chunk K=8
OLAST = VW - 4 * P             # 8

_CACHED = {}


def _chunk_k(c):
    return KLAST if c == NCH - 1 else P


def build_bass():
    nc = bacc.Bacc("TRN2", target_bir_lowering=False, debug=False, num_devices=8)
    hst = nc.dram_tensor("hst", [NCH, P, S], F32, kind="ExternalInput").ap()
    maskf = nc.dram_tensor("maskf", [P, NT], F32, kind="ExternalInput").ap()
    wqk = nc.dram_tensor("wqk", [NCH, P, 2 * QW], F32, kind="ExternalInput").ap()
    wv = nc.dram_tensor("wv", [NCH, P, VW], BF16, kind="ExternalInput").ap()
    wo = nc.dram_tensor("wo", [OCH, P, D], BF16, kind="ExternalInput").ap()
    id16 = nc.dram_tensor("id16", [P, P], BF16, kind="ExternalInput").ap()
    out = nc.dram_tensor("out", [S, D], F32, kind="ExternalOutput").ap()

    ACT = mybir.ActivationFunctionType.Copy

    with tile.TileContext(nc) as tc:
        with (
            tc.tile_pool(name="consts", bufs=1) as consts,
            tc.tile_pool(name="state", bufs=1) as state,
            tc.tile_pool(name="rot", bufs=2) as rot,
            tc.tile_pool(name="ps", bufs=1, space="PSUM") as ps,
            tc.tile_pool(name="ps2", bufs=2, space="PSUM") as ps2,
        ):
            # ---- small consts + hs^T prefetch BEFORE the big weight DMAs ----
            hstv = hst.rearrange("c p n -> p c n")
            hs_tiles = [consts.tile([P, NCH, P], F32, name=f"hstt{i}")
                        for i in range(3)]
            nc.sync.dma_start(out=hs_tiles[0], in_=hstv[:, :, 0:P])
            id16_sb = consts.tile([P, P], BF16)
            nc.sync.dma_start(out=id16_sb, in_=id16)
            mask_sb = consts.tile([P, NT], F32)
            nc.sync.dma_start(out=mask_sb, in_=maskf)

            # weights as per-chunk tiles: independent deps, so tile 0's
            # matmuls start as soon as chunk 0's DMA + split land
            wqk_sb = [consts.tile([P, 2 * QW], F32, name=f"wqk{c}")
                      for c in range(NCH)]
            wqkr = [consts.tile([P, 2 * QW], F32R, name=f"wqkr{c}")
                    for c in range(NCH)]
            wqkl = [consts.tile([P, 2 * QW], F32R, name=f"wqkl{c}")
                    for c in range(NCH)]
            wv_sb = [consts.tile([P, VW], BF16, name=f"wv{c}")
                     for c in range(NCH)]
            for c in range(NCH):
                nc.sync.dma_start(out=wqk_sb[c], in_=wqk[c])
                if c == 0:
                    nc.sync.dma_start(out=wv_sb[0], in_=wv[0])
                nc.vector.tensor_copy(wqkr[c][:], wqk_sb[c][:])
                nc.vector.tensor_tensor(wqkl[c][:], wqk_sb[c][:],
                                        wqkr[c][:].bitcast(F32),
                                        mybir.AluOpType.subtract)
            for c in range(1, NCH):
                nc.sync.dma_start(out=wv_sb[c], in_=wv[c])
            for i in (1, 2):
                nc.sync.dma_start(out=hs_tiles[i],
                                  in_=hstv[:, :, i * P:(i + 1) * P])
            wo_sb = consts.tile([P, OCH, D], BF16)
            nc.sync.dma_start(out=wo_sb, in_=wo.rearrange("c p j -> p c j"))

            half_col = consts.tile([P, 1], F32)
            nc.vector.memset(half_col[:], 0.5)
            ones_row = consts.tile([1, P], F32)
            nc.vector.memset(ones_row[:], 1.0)

            # ---- persistent state ----
            phiq = state.tile([P, NT, HPC, E], BF16)   # rq-folded phi(q) stash
            kvs_sb = state.tile([E, HPC * E], BF16)    # rksum-scaled kv
            rk_row = state.tile([1, HPC * E], F32)
            # exact per-chunk row-sums of hs^T (for the exact ksum-linear path)
            hsTsum = state.tile([P, NCH], F32)
            ksq_acc = state.tile([P, QW], F32)
            kv_ps = [ps.tile([E, VH], F32, tag=f"kv{i}", name=f"kv{i}")
                     for i in range(2)]
            # phik/v16 live one extra tile (kv deferral) -> explicit 2 bufs
            phik_t = [state.tile([P, HPC, E], BF16, name=f"phik{i}")
                      for i in range(2)]
            v16_t = [state.tile([P, VW], BF16, name=f"v16{i}")
                     for i in range(2)]
            # ones column of phi_k never changes: set once per buffer
            for i in range(2):
                nc.vector.memset(phik_t[i][:, :, 0:1], 1.0)

            def kv_mm(t):
                pk, vv = phik_t[t % 2], v16_t[t % 2]
                for h in range(HPC):
                    nc.tensor.matmul(
                        kv_ps[h // 4][:, (h % 4) * E:(h % 4) * E + E],
                        pk[:, h, :], vv[:, h * E:h * E + E],
                        start=(t == 0 and h % 4 == 0), stop=(t == NT - 1),
                        skip_group_check=True)

            # =============== PASS A ===============
            for t in range(NT):
                with nc.named_scope(f"A{t}"):
                    hs_t = hs_tiles[t % 3]

                    GW = (4, 4, 1)
                    hsr = [rot.tile([P, GW[g], P], F32R, tag=f"hsr{g}",
                                    name=f"hsr{g}_{t}") for g in range(3)]
                    hlo = [rot.tile([P, GW[g], P], F32R, tag=f"hlo{g}",
                                    name=f"hlo{g}_{t}") for g in range(3)]
                    hsT16 = [rot.tile([P, GW[g], P], BF16, tag=f"hsT16{g}",
                                      name=f"hsT16{g}_{t}") for g in range(3)]
                    for g, cs in enumerate([range(0, 4), range(4, 8), range(8, 9)]):
                        lo, hi = cs[0], cs[-1] + 1
                        nch = hi - lo
                        src = hs_t[:, lo:hi, :].rearrange("p c n -> p (c n)")
                        hr = hsr[g][:].rearrange("p c n -> p (c n)")
                        nc.scalar.activation(hr, src, ACT)
                        nc.vector.tensor_tensor(
                            hlo[g][:].rearrange("p c n -> p (c n)"),
                            src, hr.bitcast(F32), mybir.AluOpType.subtract)
                        nc.scalar.activation(
                            hsT16[g][:].rearrange("p c n -> p (c n)"),
                            src, ACT)
                        # exact hs^T row-sums (fp32) for the ksum-linear path
                        red = rot.tile([P, 4], F32, tag="hred", bufs=3,
                                       name=f"red_{t}_{g}")
                        nc.vector.tensor_reduce(
                            red[:, 0:nch], hs_t[:, lo:hi, :],
                            mybir.AxisListType.X, mybir.AluOpType.add)
                        if t == 0:
                            nc.vector.tensor_copy(hsTsum[:, lo:hi],
                                                  red[:, 0:nch])
                        else:
                            nc.vector.tensor_add(hsTsum[:, lo:hi],
                                                 hsTsum[:, lo:hi],
                                                 red[:, 0:nch])

                    # refill this hs slot for tile t+3 (readers above queued)
                    if t + 3 < NT:
                        nc.sync.dma_start(
                            out=hs_t,
                            in_=hstv[:, :, (t + 3) * P:(t + 4) * P])

                    # projections: q|k via exact 3-term fp32r, v via bf16
                    qk_ps = ps2.tile([P, 2 * QW], F32, tag="qk", name=f"qk_{t}")
                    v1_ps = ps.tile([P, VH], F32, tag="v1", name=f"v1_{t}")
                    v2_ps = ps.tile([P, VH], F32, tag="v2", name=f"v2_{t}")
                    # r-pass covers q|k (512); l/lo correction passes cover
                    # only the q half (256): k's error propagates relatively
                    # through 1/ksum (no catastrophic cancellation there).
                    for c in range(NCH):
                        kk = P
                        if c < NCH - 1:
                            nc.tensor.matmul(
                                qk_ps[:], hsr[c // 4][0:kk, c % 4, :], wqkr[c][0:kk, :],
                                start=(c == 0), stop=False,
                                skip_group_check=True)
                        else:
                            # split last chunk so the k region gets its stop
                            nc.tensor.matmul(
                                qk_ps[:, 0:QW], hsr[c // 4][0:kk, c % 4, :],
                                wqkr[c][0:kk, 0:QW],
                                start=False, stop=False,
                                skip_group_check=True)
                            nc.tensor.matmul(
                                qk_ps[:, QW:2 * QW], hsr[c // 4][0:kk, c % 4, :],
                                wqkr[c][0:kk, QW:2 * QW],
                                start=False, stop=True,
                                skip_group_check=True)
                        nc.tensor.matmul(
                            v1_ps[:], hsT16[c // 4][0:kk, c % 4, :], wv_sb[c][0:kk, 0:VH],
                            start=(c == 0), stop=(c == NCH - 1))
                        nc.tensor.matmul(
                            v2_ps[:], hsT16[c // 4][0:kk, c % 4, :], wv_sb[c][0:kk, VH:VW],
                            start=(c == 0), stop=(c == NCH - 1))
                    for c in range(NCH):
                        kk = P
                        nc.tensor.matmul(
                            qk_ps[:, 0:QW], hsr[c // 4][0:kk, c % 4, :],
                            wqkl[c][0:kk, 0:QW],
                            start=False, stop=False, skip_group_check=True)
                        nc.tensor.matmul(
                            qk_ps[:, 0:QW], hlo[c // 4][0:kk, c % 4, :],
                            wqkr[c][0:kk, 0:QW],
                            start=False, stop=(c == NCH - 1),
                            skip_group_check=True)

                    # kv for the PREVIOUS tile (its phik/v16 are long ready,
                    # so the PE never blocks on the vector chain below)
                    if t > 0:
                        kv_mm(t - 1)

                    # exact fp32 copies + squares
                    qkf = rot.tile([P, 2 * QW], F32, tag="qkf")
                    nc.scalar.activation(qkf[:], qk_ps[:], ACT)
                    qf32 = qkf[:, 0:QW]
                    kf32 = qkf[:, QW:2 * QW]
                    sq2 = rot.tile([P, QW], F32, tag="sq2")
                    nc.vector.tensor_mul(sq2[:], qk_ps[:, 0:QW], qf32)
                    sk2 = rot.tile([P, QW], F32, tag="sk2")
                    nc.vector.tensor_mul(sk2[:], qk_ps[:, QW:2 * QW], kf32)

                    # ksum-sq accumulator (per-partition partial sums, fp32)
                    if t == 0:
                        nc.vector.tensor_copy(ksq_acc[:], sk2[:])
                    else:
                        nc.vector.tensor_add(ksq_acc[:], ksq_acc[:], sk2[:])

                    # qsum = 1 + sum(q) + 0.5*sum(q^2); rq = mask/qsum
                    sumq = rot.tile([P, HPC], F32, tag="sumq")
                    nc.vector.tensor_reduce(
                        sumq[:], qf32.rearrange("p (h f) -> p h f", f=F),
                        mybir.AxisListType.X, mybir.AluOpType.add)
                    sumq2 = rot.tile([P, HPC], F32, tag="sumq2")
                    nc.vector.tensor_reduce(
                        sumq2[:], sq2[:].rearrange("p (h f) -> p h f", f=F),
                        mybir.AxisListType.X, mybir.AluOpType.add)
                    qsum = rot.tile([P, HPC], F32, tag="qsum")
                    nc.vector.tensor_scalar(
                        qsum[:], sumq2[:], 0.5, 1.0,
                        mybir.AluOpType.mult, mybir.AluOpType.add)
                    nc.vector.tensor_add(qsum[:], qsum[:], sumq[:])
                    rq = rot.tile([P, HPC], F32, tag="rq")
                    nc.vector.reciprocal(rq[:], qsum[:])
                    nc.vector.tensor_mul(
                        rq[:], rq[:], mask_sb[:, t:t + 1].broadcast_to([P, HPC]))
                    rq05 = rot.tile([P, HPC], F32, tag="rq05")
                    nc.vector.tensor_scalar_mul(rq05[:], rq[:], 0.5)

                    # phi_q (rq folded) -> stash (bf16)
                    pq = phiq[:, t]                      # [P, HPC, E]
                    nc.vector.tensor_copy(pq[:, :, 0:1], rq[:].unsqueeze(2))
                    nc.vector.tensor_mul(
                        pq[:, :, 1:1 + F],
                        qf32.rearrange("p (h f) -> p h f", f=F),
                        rq[:].unsqueeze(2).broadcast_to([P, HPC, F]))
                    nc.vector.tensor_mul(
                        pq[:, :, 1 + F:E],
                        sq2[:].rearrange("p (h f) -> p h f", f=F),
                        rq05[:].unsqueeze(2).broadcast_to([P, HPC, F]))

                    # phi_k (bf16, ones col preset) and v (bf16)
                    pk = phik_t[t % 2]
                    nc.scalar.activation(
                        pk[:, :, 1:1 + F],
                        kf32.rearrange("p (h f) -> p h f", f=F), ACT)
                    nc.vector.tensor_scalar_mul(
                        pk[:, :, 1 + F:E],
                        sk2[:].rearrange("p (h f) -> p h f", f=F), 0.5)
                    v16 = v16_t[t % 2]
                    nc.scalar.activation(v16[:, 0:VH], v1_ps[:], ACT)
                    nc.scalar.activation(v16[:, VH:VW], v2_ps[:], ACT)

            kv_mm(NT - 1)

            # =============== MID: ksum assembly (matmul-based) ===============
            with nc.named_scope("mid"):
                # [1, 512] on partition 0: exact (sum_n hs) @ Wk | 0.5*sum(k^2)
                sums_ps = ps2.tile([1, 512], F32, tag="tps", name="sums_ps")
                for c in range(NCH):
                    kk = P
                    nc.tensor.matmul(sums_ps[:, 0:QW], hsTsum[0:kk, c:c + 1],
                                     wqk_sb[c][0:kk, QW:2 * QW],
                                     start=(c == 0), stop=(c == NCH - 1),
                                     skip_group_check=True)
                nc.tensor.matmul(sums_ps[:, QW:2 * QW], half_col[:], ksq_acc[:],
                                 start=True, stop=True, skip_group_check=True)

                rk_view = rk_row[:].rearrange("o (h e) -> o h e", e=E)
                nc.vector.memset(rk_view[:, :, 0:1], float(S))
                nc.vector.tensor_copy(
                    rk_view[:, :, 1:1 + F],
                    sums_ps[:, 0:QW].rearrange("o (h f) -> o h f", f=F))
                nc.vector.tensor_copy(
                    rk_view[:, :, 1 + F:E],
                    sums_ps[:, QW:2 * QW].rearrange("o (h f) -> o h f", f=F))
                nc.vector.reciprocal(rk_row[:], rk_row[:])

                # broadcast rk over 65 partitions via PE, then scale kv
                rk_sb = state.tile([E, HPC * E], F32)
                for i in range(2):
                    rk_ps = ps.tile([E, VH], F32, tag=f"v{i + 1}",
                                    name=f"rk_ps{i}")
                    nc.tensor.matmul(rk_ps[:], ones_row[:, 0:E],
                                     rk_row[:, i * VH:(i + 1) * VH],
                                     start=True, stop=True,
                                     skip_group_check=True)
                    nc.scalar.activation(rk_sb[:, i * VH:(i + 1) * VH],
                                         rk_ps[:], ACT)
                    nc.vector.tensor_mul(
                        kvs_sb[:, i * VH:(i + 1) * VH],
                        kv_ps[i][:], rk_sb[:, i * VH:(i + 1) * VH])

            # =============== PASS B (3-deep software pipeline) ===============
            # stages for tile t: T=phiq transpose, O=o matmuls, R=o transpose,
            # W=wo matmuls. body(t) issues T(t+1) O(t) R(t-1) W(t-2).
            phiT_sbs, o_sbs, oT_sbs = {}, {}, {}

            def stage_T(t):                      # phiq -> phiT_sb [E, HPC, P]
                tp = ps2.tile([E, HPC, P], BF16, tag="tps", name=f"ptp_{t}",
                              bufs=2)
                for h in range(HPC):
                    nc.tensor.matmul(tp[:, h, :], phiq[:, t, h, :],
                                     id16_sb[:], is_transpose=True)
                phiT = rot.tile([E, HPC, P], BF16, tag="phiT",
                                name=f"phiT_{t}")
                nc.scalar.activation(
                    phiT[:].rearrange("p h n -> p (h n)"),
                    tp[:].rearrange("p h n -> p (h n)"), ACT)
                phiT_sbs[t] = phiT

            def stage_O(t):                      # o = phiT^T @ kvs [P, VW]
                phiT = phiT_sbs.pop(t)
                o_ps = [ps.tile([P, VH], F32, tag="v1", name=f"ops0_{t}"),
                        ps.tile([P, VH], F32, tag="v2", name=f"ops1_{t}")]
                for h in range(HPC):
                    nc.tensor.matmul(
                        o_ps[h // 4][:, (h % 4) * E:(h % 4) * E + E],
                        phiT[:, h, :], kvs_sb[:, h * E:h * E + E],
                        start=(h % 4 == 0), stop=(h % 4 == 3),
                        skip_group_check=True)
                o_sb = rot.tile([P, VW], BF16, tag="osb", name=f"osb_{t}")
                nc.vector.tensor_copy(o_sb[:, 0:VH], o_ps[0][:])
                nc.scalar.activation(o_sb[:, VH:VW], o_ps[1][:], ACT)
                o_sbs[t] = o_sb

            def stage_R(t):                      # o^T -> oT_sb [P, OCH, P]
                o_sb = o_sbs.pop(t)
                tp = ps.tile([P, OCH, P], BF16, tag="kv1", name=f"otp_{t}")
                for c in range(OCH):
                    kk = OLAST if c == OCH - 1 else P
                    nc.tensor.matmul(tp[0:kk, c, :],
                                     o_sb[:, c * P:c * P + kk],
                                     id16_sb[:], is_transpose=True)
                oT = rot.tile([P, OCH, P], BF16, tag="oT", name=f"oT_{t}")
                nc.vector.tensor_copy(
                    oT[:].rearrange("p c n -> p (c n)"),
                    tp[:].rearrange("p c n -> p (c n)"))
                oT_sbs[t] = oT

            def stage_W(t):                      # out = o^T.T @ Wo -> DMA
                oT = oT_sbs.pop(t)
                f1 = ps2.tile([P, 512], F32, tag="qk", name=f"f1_{t}")
                f2 = ps2.tile([P, 512], F32, tag="qk", name=f"f2_{t}")
                f3 = ps.tile([P, D - 1024], F32, tag="kv0", name=f"f3_{t}")
                for c in range(OCH):
                    kk = OLAST if c == OCH - 1 else P
                    nc.tensor.matmul(f1[:], oT[0:kk, c, :],
                                     wo_sb[0:kk, c, 0:512],
                                     start=(c == 0), stop=(c == OCH - 1))
                    nc.tensor.matmul(f2[:], oT[0:kk, c, :],
                                     wo_sb[0:kk, c, 512:1024],
                                     start=(c == 0), stop=(c == OCH - 1))
                    nc.tensor.matmul(f3[:], oT[0:kk, c, :],
                                     wo_sb[0:kk, c, 1024:D],
                                     start=(c == 0), stop=(c == OCH - 1))
                out_sb = rot.tile([P, D], F32, tag="outsb", name=f"outsb_{t}")
                nc.vector.tensor_copy(out_sb[:, 0:512], f1[:])
                nc.sync.dma_start(out=out[t * P:(t + 1) * P, 0:512],
                                  in_=out_sb[:, 0:512])
                nc.scalar.activation(out_sb[:, 512:1024], f2[:], ACT)
                nc.vector.tensor_copy(out_sb[:, 1024:D], f3[:])
                nc.sync.dma_start(out=out[t * P:(t + 1) * P, 512:D],
                                  in_=out_sb[:, 512:D])

            for b in range(NT + 3):
                with nc.named_scope(f"B{b}"):
                    if b < NT:
                        stage_T(b)
                    if 0 <= b - 1 < NT:
                        stage_O(b - 1)
                    if 0 <= b - 2 < NT:
                        stage_R(b - 2)
                    if 0 <= b - 3 < NT:
                        stage_W(b - 3)

    nc.compile()
    return nc


def _prep_core_inputs(hidden_states, attention_mask, Wq, Wk, Wv, Wo, core):
    b, half = core // 2, core % 2
    h0 = half * HPC
    bf = ml_dtypes.bfloat16

    hsT = hidden_states[b].astype(np.float32).T   # [D, S]
    hst = np.zeros((NCH, P, S), dtype=np.float32)
    for c in range(NCH):
        kk = _chunk_k(c)
        hst[c, 0:kk] = hsT[c * P:c * P + kk]
    maskf = np.ascontiguousarray(
        attention_mask[b].astype(np.float32).reshape(NT, P).T)

    def chunks(w):
        out = np.zeros((NCH, P, w.shape[1]), dtype=np.float32)
        for c in range(NCH):
            kk = _chunk_k(c)
            out[c, 0:kk] = w[c * P:c * P + kk]
        return out

    wq_h = Wq[:, h0 * F:(h0 + HPC) * F].astype(np.float32)
    wk_h = Wk[:, h0 * F:(h0 + HPC) * F].astype(np.float32)
    wqk_h = chunks(np.concatenate([wq_h, wk_h], axis=1))
    wv_h = chunks(Wv[:, h0 * E:(h0 + HPC) * E].astype(np.float32)).astype(bf)
    wo_rows = Wo[h0 * E:(h0 + HPC) * E].astype(np.float32)
    wo_h = np.zeros((OCH, P, D), dtype=np.float32)
    for c in range(OCH):
        kk = OLAST if c == OCH - 1 else P
        wo_h[c, 0:kk] = wo_rows[c * P:c * P + kk]
    wo_h = wo_h.astype(bf)

    return {
        "hst": hst,
        "maskf": maskf,
        "wqk": wqk_h,
        "wv": wv_h,
        "wo": wo_h,
        "id16": np.eye(P, dtype=np.float32).astype(bf),
    }


def kernel(hidden_states, attention_mask, Wq, Wk, Wv, Wo, _trace=False):
    hidden_states = np.asarray(hidden_states)
    attention_mask = np.asarray(attention_mask)
    Wq = np.asarray(Wq); Wk = np.asarray(Wk)
    Wv = np.asarray(Wv); Wo = np.asarray(Wo)

    if "nc" not in _CACHED:
        _CACHED["nc"] = build_bass()
    nc = _CACHED["nc"]

    in_maps = [
        _prep_core_inputs(hidden_states, attention_mask, Wq, Wk, Wv, Wo, c)
        for c in range(8)
    ]
    res = run_bass_kernel_spmd(nc, in_maps, core_ids=list(range(8)),
                               trace=_trace)
    _CACHED["last_result"] = res
    out = np.empty((B, S, D), dtype=np.float32)
    for b in range(B):
        out[b] = res.results[2 * b]["out"] + res.results[2 * b + 1]["out"]
    return out
